# revision 2
# baseline (speedup 1.0000x reference)
"""Trainium2 Bass kernel v2: batched Kabsch-aligned masked MSE.

Math: per-sample loss = (|Pc|^2+|Qc|^2 - 2 t)/(3n) with t = s1+s2+sign(detH)*s3,
s_i = singular values of the 3x3 cross-covariance H = Pc^T Qc.  s_i^2 are the
eigenvalues of K = H^T H, found in closed form (Cardano / trigonometric method
using Arctan+Sin on the ACT engine).  No eigenvector needed.

Layout: samples sorted by valid length, striped over 8 cores; on-core 32 tiles
of 128 samples (samples on partitions), pairs of tiles merged (shared length
crop L).  Inputs are bf16, zero-padded on the host, shipped pre-transposed
as [P_A | Q_A | P_B | Q_B] per partition row, in a handful of large
contiguous DMAs.  Phase 1 computes per-sample sums (H, sp, sq, sppqq) with
DVE bf16 2x products + Pool folds + DVE reduces + ACT square-accum.  Phase 2
solves the 3x3 eigenproblem elementwise on [128, C] stat tiles.
"""

import os
import numpy as np
import ml_dtypes

import bass_rust
import concourse.bass as bass
import concourse.tile as tile
from concourse import mybir
from concourse.bass_utils import run_bass_kernel_spmd

F32 = mybir.dt.float32
BF16 = mybir.dt.bfloat16
Alu = mybir.AluOpType
Act = mybir.ActivationFunctionType
AX = mybir.AxisListType

N_CORES = 8
B_FULL = 32768
N_SEQ = 128
B_CORE = B_FULL // N_CORES      # 4096
N_TILES = B_CORE // 128         # 32 sub-tiles
GM = 4                          # sub-tiles per merged tile
N_MERGED = N_TILES // GM        # merged tiles
SQ3 = 1.7320508075688772
PI = 3.141592653589793


def _legalize_single_wait(nc):
    """Split multi-wait instructions into chains of single-wait Drains
    (deployed walrus build allows only one sync-wait per instruction)."""
    moved = 0
    for fn in nc.m.functions:
        for blk in fn.blocks:
            insts = blk.instructions
            new_list = []
            for ins in insts:
                si = ins.sync_info
                ow = list(si.on_wait) if si is not None and si.on_wait else []
                if len(ow) > 1:
                    for w in ow[:-1]:
                        d = mybir.InstDrain(name=f"I-sw{moved}", ins=[],
                                            outs=[], bass_is_fusable=False)
                        d.engine = ins.engine
                        d.sync_info = bass_rust.SyncInfo(on_wait=[w],
                                                         on_update=[])
                        new_list.append(d)
                        moved += 1
                    si.on_wait = [ow[-1]]
                new_list.append(ins)
            blk.instructions[:] = new_list
    return moved


def _ap(base, extra_offset, dims):
    """Manual AP: keep base's partition dim, replace free dims."""
    return bass.AP(tensor=base.tensor, offset=base.offset + extra_offset,
                   ap=[base.ap[0]] + [list(d) for d in dims])


def _emit_products(tc, pools, in_sb, m, L, off):
    """Products for merged tile m -> bf16 tile [p, 18, L], g-major blocks."""
    nc = tc.nc
    V = nc.vector
    prod = pools["work"].tile([128, 9 * GM * 128], BF16, tag="prod",
                              name="prod")
    for g in range(GM):
        p0 = off + 6 * L * g
        Pv = (in_sb[:, p0:p0 + 3 * L]
              .rearrange("p (i n) -> p i n", i=3)
              .unsqueeze(2).broadcast_to([128, 3, 3, L]))
        Qv = (in_sb[:, p0 + 3 * L:p0 + 6 * L]
              .rearrange("p (j n) -> p j n", j=3)
              .unsqueeze(1).broadcast_to([128, 3, 3, L]))
        out = prod[:, 9 * L * g:9 * L * (g + 1)].rearrange(
            "p (i j n) -> p i j n", i=3, j=3)
        V.tensor_tensor(out=out, in0=Pv, in1=Qv, op=Alu.mult)
    return prod


def _phase1_rest(tc, pools, in_sb, st, m, L, off, prod):
    """Folds + reduce + sppqq for merged tile m.

    Combined fold buffer blocks (30 x L2): [H_A(9) H_B(9) c_A(6) c_B(6)];
    three fold levels, then two TRs write st['all'][:, 2m:2m+2, :]
    (per sub-tile 15 = H(9), sp(3), sq(3)).
    """
    nc = tc.nc
    V, G, A = nc.vector, nc.gpsimd, nc.scalar
    L2, L4, L8 = L // 2, L // 4, L // 8
    bH = pools.get("bH", 28)    # of 9*GM H-fold blocks on Pool
    bC = pools.get("bC", 16)    # of 6*GM c-fold blocks on Pool
    if L <= pools.get("poolmin", 0):
        bH = bC = 0             # short tiles: avoid cross-engine latency

    NB = 15 * GM
    NH = 9 * GM
    NC = 6 * GM
    fb = pools["work"].tile([128, NB * 64], BF16, tag="fold", name="fold")
    fb2 = pools["work"].tile([128, NB * 32], BF16, tag="fold2", name="fold2")
    fb3 = pools["work"].tile([128, NB * 16], BF16, tag="fold3", name="fold3")
    ascr = pools["scr"].tile([128, 6 * 128], BF16, tag="ascr", name="ascr")

    fv = fb[:, 0:NB * L2].rearrange("p (k n) -> p k n", k=NB)
    fv2 = fb2[:, 0:NB * L4].rearrange("p (k n) -> p k n", k=NB)
    fv3 = fb3[:, 0:NB * L8].rearrange("p (k n) -> p k n", k=NB)
    pv = prod[:, 0:NH * L].rearrange("p (k n) -> p k n", k=NH)
    iv = in_sb[:, off:off + NC * L].rearrange("p (k n) -> p k n", k=NC)

    # fold1: H blocks [0,NH) from prod, c blocks [NH,NB) from input;
    # first bH/bC blocks on Pool, rest on DVE
    for dst0, srcv, nblk, npool in ((0, pv, NH, bH), (NH, iv, NC, bC)):
        for eng, k0, k1 in ((G, 0, npool), (V, npool, nblk)):
            if k0 >= k1:
                continue
            eng.tensor_tensor(
                out=fv[:, dst0 + k0:dst0 + k1, :],
                in0=srcv[:, k0:k1, 0:L2],
                in1=srcv[:, k0:k1, L2:2 * L2],
                op=Alu.add)

    # extra fold levels while profitable (halving pays iff width/2 >= 4)
    last = fv
    width = L2
    for nxt in (fv2, fv3):
        if width // 2 < 4:
            break
        w2 = width // 2
        V.tensor_tensor(out=nxt[:, :, 0:w2], in0=last[:, :, 0:w2],
                        in1=last[:, :, w2:width], op=Alu.add)
        last, width = nxt, w2
    st3 = st["all"][:, :, :]
    outH = _ap(st3, 15 * GM * m, [[15, GM], [1, 9]])
    V.tensor_reduce(out=outH, in_=last[:, 0:NH, 0:width], axis=AX.X,
                    op=Alu.add)
    outC = _ap(st3, 15 * GM * m + 9, [[15, GM], [1, 6]])
    V.tensor_reduce(out=outC, in_=last[:, NH:NB, 0:width], axis=AX.X,
                    op=Alu.add)

    # sppqq per sub-tile: ACT square with accumulate over [p, 6L]
    for g in range(GM):
        p0 = off + 6 * L * g
        t = GM * m + g
        A.activation(out=ascr[:, 0:6 * L], in_=in_sb[:, p0:p0 + 6 * L],
                     func=Act.Square,
                     accum_out=st["ss"][:, t:t + 1])


class P2:
    """Emit elementwise phase-2 ops on [128, C] column tiles."""

    def __init__(self, tc, pool, c0, c1, chunk, dma_out=None):
        self.nc = tc.nc
        self.pool = pool
        self.c0, self.c1 = c0, c1
        self.C = c1 - c0
        self.chunk = chunk
        self.ctr = 0
        self.dma_out = dma_out

    def mk(self, name=None):
        self.ctr += 1
        tag = f"c{self.chunk}_" + (name or f"t{self.ctr}")
        return self.pool.tile([128, self.C], F32, tag=tag, name=tag)

    def tt(self, a, b, op, eng=None, out=None):
        dst = out if out is not None else self.mk()
        (eng or self.nc.vector).tensor_tensor(out=dst, in0=a, in1=b, op=op)
        return dst

    def mul(self, a, b, eng=None, out=None):
        return self.tt(a, b, Alu.mult, eng, out)

    def add(self, a, b, eng=None, out=None):
        return self.tt(a, b, Alu.add, eng, out)

    def sub(self, a, b, eng=None, out=None):
        return self.tt(a, b, Alu.subtract, eng, out)

    def ts(self, a, s1, op0, s2=None, op1=Alu.bypass, eng=None, out=None):
        dst = out if out is not None else self.mk()
        (eng or self.nc.vector).tensor_scalar(
            out=dst, in0=a, scalar1=s1, scalar2=s2, op0=op0, op1=op1)
        return dst

    def stt(self, a, s, b, op0, op1, eng=None, out=None):
        """(a op0 s) op1 b in one instruction."""
        dst = out if out is not None else self.mk()
        (eng or self.nc.vector).scalar_tensor_tensor(
            out=dst, in0=a, scalar=s, in1=b, op0=op0, op1=op1)
        return dst

    def recip(self, a, out=None):
        dst = out if out is not None else self.mk()
        self.nc.vector.reciprocal(out=dst, in_=a)
        return dst

    def act(self, a, func, bias=0.0, scale=1.0, out=None):
        dst = out if out is not None else self.mk()
        self.nc.scalar.activation(out=dst, in_=a, func=func, bias=bias,
                                  scale=scale)
        return dst


def _phase2(tc, p2, st, cst, loss_out):
    """Per-sample Kabsch loss from stats, columns [c0, c1) (c = sub-tile).

    t = lam + 4*wx(lam)/p'(lam); lam = s1+s2+d*s3 via Cardano on K = Hc^T Hc;
    p'(lam) = 8(s2+d*s3)(s1+d*s3)(s1+s2); wx(lam) = -|a|^2 lam^2 + Wb lam + Wc
    is the adjugate-row-0 dot product, coefficients lam-free (computed early,
    off the critical path).
    """
    nc = tc.nc
    V, G, A = nc.vector, nc.gpsimd, nc.scalar
    c0, C = p2.c0, p2.C

    St = st["all"][:, :, :]        # [p, 32, 15]
    H9 = _ap(St, 15 * c0, [[15, C], [3, 3], [1, 3]])     # [p, c, i, j]
    sp_b = _ap(St, 15 * c0 + 9, [[15, C], [1, 3], [0, 3]])
    ss = st["ss"][:, c0:c0 + C]
    invn = cst[:, c0:c0 + C]
    invn3 = cst[:, N_TILES + c0:N_TILES + c0 + C]

    def wide(name, k):
        tag = f"c{p2.chunk}_{name}"
        return p2.pool.tile([128, C * k], F32, tag=tag, name=tag)

    # spqn = spq * invn (6-wide); corr = sum(spq*spqn); ppqqc = ss - corr
    spq6 = _ap(St, 15 * c0 + 9, [[15, C], [1, 6]])
    spqn6 = wide("spqn6", 6)
    spqn6_v = spqn6[:, :].rearrange("p (c k) -> p c k", k=6)
    inb6 = invn[:, :].unsqueeze(2).broadcast_to([128, C, 6])
    V.tensor_tensor(out=spqn6_v, in0=spq6, in1=inb6, op=Alu.mult)
    corrp = wide("corrp", 6)
    corrp_v = corrp[:, :].rearrange("p (c k) -> p c k", k=6)
    G.tensor_tensor(out=corrp_v, in0=spq6, in1=spqn6_v, op=Alu.mult)
    corr = p2.mk("corr")
    V.tensor_reduce(out=corr, in_=corrp_v, axis=AX.X, op=Alu.add)
    ppqqc = p2.sub(ss, corr, G)

    # centering: Hc[c, i, j] = H - sp_i * sqn_j
    mv = wide("mv", 9)
    mv_v = mv[:, :].rearrange("p (c i j) -> p c i j", i=3, j=3)
    sqn_b = bass.AP(tensor=spqn6_v.tensor, offset=spqn6_v.offset + 3,
                    ap=[spqn6_v.ap[0], [6, C], [0, 3], [1, 3]])
    V.tensor_tensor(out=mv_v, in0=sp_b, in1=sqn_b, op=Alu.mult)
    Hc = wide("Hc", 9)
    Hc_v = Hc[:, :].rearrange("p (c k) -> p c k", k=9)
    H9f = _ap(St, 15 * c0, [[15, C], [1, 9]])
    V.tensor_tensor(out=Hc_v, in0=H9f, in1=mv[:, :].rearrange(
        "p (c k) -> p c k", k=9), op=Alu.subtract)
    hc0 = Hc[:, :]
    h = {(i, j): _ap(hc0, 3 * i + j, [[9, C]]) for i in range(3)
         for j in range(3)}

    # K = Hc^T Hc: 3 products into one (c,a,b,i) tile, single reduce
    Kt = wide("Kt", 9)
    kp = wide("kp", 27)
    for aa in range(3):
        in0 = _ap(hc0, aa, [[9, C], [0, 3], [3, 3]])
        in1 = _ap(hc0, 0, [[9, C], [1, 3], [3, 3]])
        kp_v = _ap(kp[:, :], 9 * aa, [[27, C], [3, 3], [1, 3]])
        V.tensor_tensor(out=kp_v, in0=in0, in1=in1, op=Alu.mult)
    kp_flat = _ap(kp[:, :], 0, [[3, 9 * C], [1, 3]])
    V.tensor_reduce(out=Kt[:, :], in_=kp_flat, axis=AX.X, op=Alu.add)
    trK = p2.mk("trK")
    diag_v = _ap(Kt[:, :], 0, [[9, C], [4, 3]])
    V.tensor_reduce(out=trK, in_=diag_v, axis=AX.X, op=Alu.add)
    k2 = wide("k2", 9)
    V.tensor_tensor(out=k2[:, :], in0=Kt[:, :], in1=Kt[:, :], op=Alu.mult)
    trK2 = p2.mk("trK2")
    V.tensor_reduce(out=trK2, in_=k2[:, :].rearrange("p (c k) -> p c k", k=9),
                    axis=AX.X, op=Alu.add)

    # detH (of Hc) via 2x2 minors (Pool, off-spine)
    def minor2(pq, qq, rq_, sq_, eng=G):
        t1 = p2.mul(pq, qq, eng)
        t2 = p2.mul(rq_, sq_, eng)
        return p2.sub(t1, t2, eng)

    mm1 = minor2(h[(1, 1)], h[(2, 2)], h[(1, 2)], h[(2, 1)])
    mm2 = minor2(h[(1, 0)], h[(2, 2)], h[(1, 2)], h[(2, 0)])
    mm3 = minor2(h[(1, 0)], h[(2, 1)], h[(1, 1)], h[(2, 0)])
    dd1 = p2.mul(h[(0, 0)], mm1, G)
    dd2 = p2.mul(h[(0, 1)], mm2, G)
    dd3 = p2.mul(h[(0, 2)], mm3, G)
    detH = p2.add(p2.sub(dd1, dd2, G), dd3, G)
    sgn = p2.act(detH, Act.Sign)
    detK = p2.act(detH, Act.Square)

    # --- Cardano spine starts (DVE), W-coefficient work interleaved into
    # the spine's dependency-stall windows ---
    trKsq = p2.mul(trK, trK, V)
    p6 = p2.stt(trKsq, -1.0 / 3.0, trK2, Alu.mult, Alu.add, V)
    p6c = p2.ts(p6, 1e-12, Alu.max, eng=V)
    sqp = p2.act(p6c, Act.Sqrt, scale=1.0 / 6.0)         # sqrt(p)
    mmean = p2.ts(trK, 1.0 / 3.0, Alu.mult, eng=V)

    # [fill] Horn matrix entries of M = Hc^T, packed for one-shot squares
    npk1 = p2.pool.tile([128, 3 * C], F32, tag=f"c{p2.chunk}_npk1",
                        name="npk1")
    npk2 = p2.pool.tile([128, 3 * C], F32, tag=f"c{p2.chunk}_npk2",
                        name="npk2")
    n01 = p2.sub(h[(2, 1)], h[(1, 2)], G, out=npk1[:, 0:C])
    n02 = p2.sub(h[(0, 2)], h[(2, 0)], G, out=npk1[:, C:2 * C])
    n03 = p2.sub(h[(1, 0)], h[(0, 1)], G, out=npk1[:, 2 * C:3 * C])
    n23 = p2.add(h[(2, 1)], h[(1, 2)], V, out=npk2[:, 0:C])
    n13 = p2.add(h[(0, 2)], h[(2, 0)], V, out=npk2[:, C:2 * C])
    n12 = p2.add(h[(1, 0)], h[(0, 1)], V, out=npk2[:, 2 * C:3 * C])

    msq = p2.ts(trKsq, 1.0 / 9.0, Alu.mult, eng=V)
    m3c = p2.mul(msq, mmean, G)
    u = p2.stt(detK, 0.5, m3c, Alu.mult, Alu.add, V)     # m^3 + detK/2
    tdiff = p2.sub(trKsq, trK2, V)                       # 2*M2
    tm = p2.mul(tdiff, mmean, V)
    q = p2.stt(tm, -0.25, u, Alu.mult, Alu.add, V)
    p6sq = p2.mul(p6c, p6c, V)
    p3 = p2.mul(p6sq, p6c, V)
    q2 = p2.mul(q, q, V)
    pfloor = p2.ts(p3, 9.26e-11, Alu.mult, eng=V)
    diff = p2.stt(p3, 1.0 / 216.0, q2, Alu.mult, Alu.subtract, V)
    diffc = p2.tt(diff, pfloor, Alu.max, V)
    sqd = p2.act(diffc, Act.Sqrt)

    # [fill] squares of the packed entries + first W terms
    usq = p2.pool.tile([128, 3 * C], F32, tag=f"c{p2.chunk}_usq", name="usq")
    A.activation(out=usq[:, :], in_=npk1[:, :], func=Act.Square)
    u1, u2, u3 = usq[:, 0:C], usq[:, C:2 * C], usq[:, 2 * C:3 * C]
    wsq = p2.pool.tile([128, 3 * C], F32, tag=f"c{p2.chunk}_wsq", name="wsq")
    A.activation(out=wsq[:, :], in_=npk2[:, :], func=Act.Square)
    n23s, n13s, n12s = wsq[:, 0:C], wsq[:, C:2 * C], wsq[:, 2 * C:3 * C]
    tr3 = p2.add(p2.add(h[(0, 0)], h[(1, 1)], V), h[(2, 2)], V)
    n11 = p2.stt(h[(0, 0)], 2.0, tr3, Alu.mult, Alu.subtract, V)
    n22 = p2.stt(h[(1, 1)], 2.0, tr3, Alu.mult, Alu.subtract, V)
    n33 = p2.stt(h[(2, 2)], 2.0, tr3, Alu.mult, Alu.subtract, V)
    v1 = p2.mul(n01, n02, G)
    v2 = p2.mul(n01, n03, G)
    v3 = p2.mul(n02, n03, G)

    rq = p2.recip(sqd)
    ratio = p2.mul(q, rq, V)
    ratioc = p2.ts(ratio, 100.0, Alu.min, -100.0, Alu.max, V)
    at = p2.act(ratioc, Act.Arctan)

    # [fill] Wa, Wb
    Wa_n = p2.add(p2.add(u1, u2, V), u3, V)
    s1s = p2.add(n22, n33, G)
    s2s = p2.add(n11, n33, G)
    s3s = p2.add(n11, n22, G)
    b1 = p2.mul(u1, s1s, V)
    b2 = p2.mul(u2, s2s, V)
    b3 = p2.mul(u3, s3s, V)
    b4 = p2.mul(v1, n12, G)
    b5 = p2.mul(v2, n13, G)
    b6 = p2.mul(v3, n23, G)

    # packed Sin: [cos(phi) | sin(phi)] in one ACT op
    scp = wide("scp", 2)
    p2.ts(at, -1.0 / 3.0, Alu.mult, PI / 6.0 + PI / 2.0, Alu.add, V,
          out=scp[:, 0:C])
    p2.ts(at, -1.0 / 3.0, Alu.mult, PI / 6.0, Alu.add, V,
          out=scp[:, C:2 * C])
    sc = p2.pool.tile([128, 2 * C], F32, tag=f"c{p2.chunk}_sc", name="sc")
    A.activation(out=sc[:, :], in_=scp[:, :], func=Act.Sin)
    cphi = sc[:, 0:C]
    sphi = sc[:, C:2 * C]

    # [fill] Wb finish, Wc terms
    a123 = p2.add(p2.add(b1, b2, V), b3, V)
    c456 = p2.add(p2.add(b4, b5, G), b6, G)
    Wb = p2.stt(c456, -2.0, a123, Alu.mult, Alu.add, V)
    M1 = p2.sub(p2.mul(n22, n33, G), n23s, G)
    M2m = p2.sub(p2.mul(n11, n33, G), n13s, G)
    M3m = p2.sub(p2.mul(n11, n22, G), n12s, G)

    # eigenvalues via mp +/- sqrt(3)*ps; one packed Sqrt for all three
    pc = p2.mul(sqp, cphi, V)
    ps = p2.mul(sqp, sphi, V)
    lamp = p2.pool.tile([128, 3 * C], F32, tag=f"c{p2.chunk}_lamp",
                        name="lamp")
    p2.stt(pc, 2.0, mmean, Alu.mult, Alu.add, V, out=lamp[:, 0:C])
    mp = p2.sub(mmean, pc, V)
    s3p = p2.ts(ps, SQ3, Alu.mult, eng=V)
    lam2 = p2.add(mp, s3p, V)
    p2.ts(lam2, 0.0, Alu.max, eng=V, out=lamp[:, C:2 * C])
    lam3 = p2.sub(mp, s3p, V)
    p2.ts(lam3, 0.0, Alu.max, eng=V, out=lamp[:, 2 * C:3 * C])
    sgt = p2.pool.tile([128, 3 * C], F32, tag=f"c{p2.chunk}_sgt", name="sgt")
    A.activation(out=sgt[:, :], in_=lamp[:, :], func=Act.Sqrt)
    sg1 = sgt[:, 0:C]
    sg2 = sgt[:, C:2 * C]
    sg3 = sgt[:, 2 * C:3 * C]

    # [fill] Wc finish
    dd_ = p2.add(p2.add(p2.mul(u1, M1, G), p2.mul(u2, M2m, G), G),
                 p2.mul(u3, M3m, G), G)
    cc1 = p2.sub(p2.mul(n12, n33, V), p2.mul(n13, n23, V), V)
    cc2 = p2.sub(p2.mul(n12, n23, V), p2.mul(n13, n22, V), V)
    cc3 = p2.sub(p2.mul(n11, n23, V), p2.mul(n12, n13, V), V)
    ee = p2.add(p2.sub(p2.mul(v1, cc1, V), p2.mul(v2, cc2, V), V),
                p2.mul(v3, cc3, V), V)
    Wc = p2.stt(ee, 2.0, dd_, Alu.mult, Alu.subtract, V)   # 2*ee - dd

    s3d = p2.mul(sgn, sg3, V)
    t12 = p2.add(sg1, sg2, V)
    lam = p2.add(t12, s3d, V)                            # lambda_max of Horn
    pp1 = p2.add(sg2, s3d, G)
    pp2 = p2.add(sg1, s3d, G)
    ppr = p2.mul(pp1, pp2, G)
    ppr2 = p2.mul(ppr, t12, G)                           # p'(lam)/8
    pprc = p2.ts(ppr2, 1e-13, Alu.max, eng=V)
    rp = p2.recip(pprc)

    # wx = (-Wa_n*lam + Wb)*lam + Wc, then t and the loss
    wt1 = p2.mul(Wa_n, lam, V)
    wt2 = p2.sub(Wb, wt1, V)
    wt3 = p2.mul(wt2, lam, V)
    wx_v = p2.add(wt3, Wc, V)
    corr4 = p2.mul(wx_v, rp, V)
    t_unc = p2.stt(corr4, 0.5, lam, Alu.mult, Alu.add, V)  # lam + 4wx/p'
    ssum = p2.add(t12, sg3, V)
    tb = p2.tt(t_unc, ssum, Alu.min, V)
    ssn = p2.ts(ssum, -1.0, Alu.mult, eng=V)
    tcl = p2.tt(tb, ssn, Alu.max, V)
    li = p2.stt(tcl, -2.0, ppqqc, Alu.mult, Alu.add, V)
    p2.mul(li, invn3, V, out=loss_out)
    if p2.dma_out is not None:
        p2.dma_out(0, C)


def build_program(lmaxes, chunks=((0, 32),), n_dma=9, bH=28, bC=16,
                  wbufs=3, order="desc"):
    """lmaxes: per-merged-tile crop lengths (16 ints, multiples of 4)."""
    assert len(lmaxes) == N_MERGED
    tot = sum(6 * GM * L for L in lmaxes)
    offs = []
    o = 0
    for L in lmaxes:
        offs.append(o)
        o += 6 * GM * L

    nc = bass.Bass("TRN2", debug=False, enable_asserts=False,
                   target_bir_lowering=False)
    # extra activation-bias constants (only 0.0/1.0 pre-registered)
    for cval in (PI / 2.0,):
        cten = nc.alloc_sbuf_tensor(f"const-f32-{cval}", [128, 1], F32)
        nc.gpsimd.memset(cten.ap(), cval)
        nc.const_aps.aps[(F32, cval)] = cten.ap()
    nc.all_engine_barrier()
    pq = nc.dram_tensor("pq", [128, tot], BF16, kind="ExternalInput").ap()
    cstd = nc.dram_tensor("cst", [128, 2 * N_TILES], F32,
                          kind="ExternalInput").ap()
    loss = nc.dram_tensor("loss", [128, N_TILES], F32,
                          kind="ExternalOutput").ap()

    with tile.TileContext(nc) as tc:
        from contextlib import ExitStack
        with ExitStack() as ctx:
            pools = {
                "in": ctx.enter_context(tc.tile_pool(name="inp", bufs=1)),
                "work": ctx.enter_context(tc.tile_pool(name="work", bufs=wbufs)),
                "scr": ctx.enter_context(tc.tile_pool(name="scr", bufs=3)),
                "stats": ctx.enter_context(tc.tile_pool(name="stats", bufs=1)),
                "ph2": ctx.enter_context(tc.tile_pool(name="ph2", bufs=1)),
            }
            pools["bH"] = bH
            pools["bC"] = bC
            in_sb = pools["in"].tile([128, tot], BF16, tag="in", name="in")
            cst = pools["stats"].tile([128, 2 * N_TILES], F32, tag="cst",
                                      name="cst")
            st = {
                "all": pools["stats"].tile([128, N_TILES, 15], F32,
                                           tag="st_all", name="st_all"),
                "ss": pools["stats"].tile([128, N_TILES], F32,
                                          tag="st_ss", name="st_ss"),
            }
            loss_tile = pools["ph2"].tile([128, N_TILES], F32, tag="loss",
                                          name="loss")

            # input DMAs: small first chunk so compute starts early, then
            # n_dma-1 even chunks over the rest; cst after the first chunk
            bounds = [0, 1]
            rem = N_MERGED - 1
            for d in range(n_dma - 1):
                bounds.append(1 + ((d + 1) * rem) // (n_dma - 1))
            first = True
            for ma, mb in zip(bounds[:-1], bounds[1:]):
                if ma >= mb:
                    continue
                e0 = offs[ma]
                e1 = offs[mb - 1] + 6 * GM * lmaxes[mb - 1]
                if first:
                    # halve the first chunk so compute can start sooner
                    eh = e0 + 3 * GM * lmaxes[ma]
                    nc.sync.dma_start(out=in_sb[:, e0:eh], in_=pq[:, e0:eh])
                    nc.sync.dma_start(out=in_sb[:, eh:e1], in_=pq[:, eh:e1])
                    nc.sync.dma_start(out=cst[:, :], in_=cstd)
                    first = False
                else:
                    nc.sync.dma_start(out=in_sb[:, e0:e1], in_=pq[:, e0:e1])

            ci = 0
            prods = {}
            prods[0] = _emit_products(tc, pools, in_sb, 0, lmaxes[0], offs[0])
            for m in range(N_MERGED):
                if m + 1 < N_MERGED:
                    prods[m + 1] = _emit_products(
                        tc, pools, in_sb, m + 1, lmaxes[m + 1], offs[m + 1])
                _phase1_rest(tc, pools, in_sb, st, m, lmaxes[m], offs[m],
                             prods.pop(m))
                while ci < len(chunks) and GM * (m + 1) >= chunks[ci][1]:
                    a, b = chunks[ci]

                    def _dma_out(x0, x1, a=a):
                        nc.sync.dma_start(out=loss[:, a + x0:a + x1],
                                          in_=loss_tile[:, a + x0:a + x1])
                    p2 = P2(tc, pools["ph2"], a, b, ci, dma_out=_dma_out)
                    _phase2(tc, p2, st, cst, loss_tile[:, a:b])
                    ci += 1
    _legalize_single_wait(nc)
    return nc


_nc_cache = {}


def _get_program(lmaxes, chunks=((0, 32),), n_dma=9, bH=28, bC=16, wbufs=3,
                 order="desc"):
    key = (lmaxes, chunks, n_dma, bH, bC, wbufs, order)
    if key not in _nc_cache:
        _nc_cache[key] = build_program(lmaxes, chunks, n_dma, bH, bC, wbufs,
                                       order)
    return _nc_cache[key]


def _prep(pred_coord, true_coord, pad_mask, torder="desc"):
    """Host-side packing. Returns (lmaxes, in_maps)."""
    P = np.asarray(pred_coord, dtype=np.float32)
    Q = np.asarray(true_coord, dtype=np.float32)
    M = np.asarray(pad_mask)
    B = P.shape[0]
    assert B == B_FULL and P.shape[1] == N_SEQ

    lengths = (N_SEQ - M.sum(axis=1)).astype(np.int64)
    order = np.argsort(lengths, kind="stable")
    lsort = lengths[order]
    # merged tile m takes sorted block blk[m]; longest first so the early
    # DMA chunks carry the most compute
    if torder == "ilv":
        blk = []
        hi, lo = N_MERGED - 1, N_MERGED // 2 - 1
        for i in range(N_MERGED // 2):
            blk.append(hi - i)
            blk.append(lo - i)
        blk = tuple(blk)
    else:
        blk = tuple(range(N_MERGED - 1, -1, -1))
    bsz = 1024 * GM
    lmaxes = []
    for m in range(N_MERGED):
        L = int(lsort[bsz * (blk[m] + 1) - 1])
        L = max(8, (L + 7) & ~7)
        lmaxes.append(L)
    lmaxes = tuple(lmaxes)

    # zero padding, transpose to [B, 3, N], sort
    w = (np.arange(N_SEQ)[None, :] < lengths[:, None]).astype(np.float32)
    Pz = (P * w[:, :, None]).transpose(0, 2, 1)[order]   # [B, 3, N]
    Qz = (Q * w[:, :, None]).transpose(0, 2, 1)[order]
    Pb = Pz.astype(ml_dtypes.bfloat16)
    Qb = Qz.astype(ml_dtypes.bfloat16)

    tot = sum(6 * GM * L for L in lmaxes)
    in_maps = []
    linv = (1.0 / lsort.astype(np.float64)).astype(np.float32)
    for c in range(N_CORES):
        buf = np.zeros((128, tot), dtype=ml_dtypes.bfloat16)
        o = 0
        for m in range(N_MERGED):
            L = lmaxes[m]
            bm = blk[m]
            gsel = np.arange(bsz * bm + c, bsz * (bm + 1), 8)  # 128*GM sorted
            Pm = Pb[gsel][:, :, :L]      # [128*GM, 3, L]
            Qm = Qb[gsel][:, :, :L]
            for g in range(GM):
                sl = slice(128 * g, 128 * (g + 1))
                buf[:, o:o + 3 * L] = Pm[sl].reshape(128, 3 * L)
                buf[:, o + 3 * L:o + 6 * L] = Qm[sl].reshape(128, 3 * L)
                o += 6 * L
        # constants: invn (32 cols), invn/3 (32 cols); col t, partition p
        # -> sorted index (t*128+p)*8 + c
        idx = (np.arange(B_CORE) * 8 + c)
        nin = linv[idx].reshape(N_TILES, 128).T          # [128, 32]
        # column t = sub-tile GM*m+g holds sorted sub-block GM*blk[m]+g
        perm = [GM * blk[t // GM] + (t % GM) for t in range(N_TILES)]
        nin = nin[:, perm]
        cstv = np.concatenate([nin, nin / 3.0], axis=1).astype(np.float32)
        in_maps.append({"pq": buf, "cst": np.ascontiguousarray(cstv)})
    return lmaxes, in_maps


def kernel(pred_coord, true_coord, pad_mask):
    lmaxes, in_maps = _prep(pred_coord, true_coord, pad_mask)
    nc = _get_program(lmaxes)
    trace = bool(int(os.environ.get("KERNEL_TRACE", "0")))
    res = run_bass_kernel_spmd(nc, in_maps, core_ids=list(range(N_CORES)),
                               trace=trace)
    if trace and res.exec_time_ns is not None:
        print(f"HW exec time: {res.exec_time_ns} ns")
        kernel.last_exec_time_ns = res.exec_time_ns
    total = 0.0
    for r in res.results:
        total += r["loss"].astype(np.float64).sum()
    return np.float32(total / B_FULL)


kernel.last_exec_time_ns = None


# revision 3
# speedup vs baseline: 1.0092x; 1.0092x over previous
"""Trainium2 Bass kernel v2: batched Kabsch-aligned masked MSE.

Math: per-sample loss = (|Pc|^2+|Qc|^2 - 2 t)/(3n) with t = s1+s2+sign(detH)*s3,
s_i = singular values of the 3x3 cross-covariance H = Pc^T Qc.  s_i^2 are the
eigenvalues of K = H^T H, found in closed form (Cardano / trigonometric method
using Arctan+Sin on the ACT engine).  No eigenvector needed.

Layout: samples sorted by valid length, striped over 8 cores; on-core 32 tiles
of 128 samples (samples on partitions), pairs of tiles merged (shared length
crop L).  Inputs are bf16, zero-padded on the host, shipped pre-transposed
as [P_A | Q_A | P_B | Q_B] per partition row, in a handful of large
contiguous DMAs.  Phase 1 computes per-sample sums (H, sp, sq, sppqq) with
DVE bf16 2x products + Pool folds + DVE reduces + ACT square-accum.  Phase 2
solves the 3x3 eigenproblem elementwise on [128, C] stat tiles.
"""

import os
import numpy as np
import ml_dtypes

import bass_rust
import concourse.bass as bass
import concourse.tile as tile
from concourse import mybir
from concourse.bass_utils import run_bass_kernel_spmd

F32 = mybir.dt.float32
BF16 = mybir.dt.bfloat16
Alu = mybir.AluOpType
Act = mybir.ActivationFunctionType
AX = mybir.AxisListType

N_CORES = 8
B_FULL = 32768
N_SEQ = 128
B_CORE = B_FULL // N_CORES      # 4096
N_TILES = B_CORE // 128         # 32 sub-tiles
GM = 4                          # sub-tiles per merged tile
N_MERGED = N_TILES // GM        # merged tiles
SQ3 = 1.7320508075688772
PI = 3.141592653589793


def _legalize_single_wait(nc):
    """Split multi-wait instructions into chains of single-wait Drains
    (deployed walrus build allows only one sync-wait per instruction)."""
    moved = 0
    for fn in nc.m.functions:
        for blk in fn.blocks:
            insts = blk.instructions
            new_list = []
            for ins in insts:
                si = ins.sync_info
                ow = list(si.on_wait) if si is not None and si.on_wait else []
                if len(ow) > 1:
                    for w in ow[:-1]:
                        d = mybir.InstDrain(name=f"I-sw{moved}", ins=[],
                                            outs=[], bass_is_fusable=False)
                        d.engine = ins.engine
                        d.sync_info = bass_rust.SyncInfo(on_wait=[w],
                                                         on_update=[])
                        new_list.append(d)
                        moved += 1
                    si.on_wait = [ow[-1]]
                new_list.append(ins)
            blk.instructions[:] = new_list
    return moved


def _ap(base, extra_offset, dims):
    """Manual AP: keep base's partition dim, replace free dims."""
    return bass.AP(tensor=base.tensor, offset=base.offset + extra_offset,
                   ap=[base.ap[0]] + [list(d) for d in dims])


def _emit_products(tc, pools, in_sb, m, L, off):
    """Products for merged tile m -> bf16 tile [p, 18, L], g-major blocks."""
    nc = tc.nc
    V = nc.vector
    prod = pools["work"].tile([128, 9 * GM * 128], BF16, tag="prod",
                              name="prod")
    for g in range(GM):
        p0 = off + 6 * L * g
        Pv = (in_sb[:, p0:p0 + 3 * L]
              .rearrange("p (i n) -> p i n", i=3)
              .unsqueeze(2).broadcast_to([128, 3, 3, L]))
        Qv = (in_sb[:, p0 + 3 * L:p0 + 6 * L]
              .rearrange("p (j n) -> p j n", j=3)
              .unsqueeze(1).broadcast_to([128, 3, 3, L]))
        out = prod[:, 9 * L * g:9 * L * (g + 1)].rearrange(
            "p (i j n) -> p i j n", i=3, j=3)
        V.tensor_tensor(out=out, in0=Pv, in1=Qv, op=Alu.mult)
    return prod


def _phase1_rest(tc, pools, in_sb, st, m, L, off, prod):
    """Folds + reduce + sppqq for merged tile m.

    Combined fold buffer blocks (30 x L2): [H_A(9) H_B(9) c_A(6) c_B(6)];
    three fold levels, then two TRs write st['all'][:, 2m:2m+2, :]
    (per sub-tile 15 = H(9), sp(3), sq(3)).
    """
    nc = tc.nc
    V, G, A = nc.vector, nc.gpsimd, nc.scalar
    L2, L4, L8 = L // 2, L // 4, L // 8
    bH = pools.get("bH", 28)    # of 9*GM H-fold blocks on Pool
    bC = pools.get("bC", 18)    # of 6*GM c-fold blocks on Pool
    if L <= pools.get("poolmin", 0):
        bH = bC = 0             # short tiles: avoid cross-engine latency

    NB = 15 * GM
    NH = 9 * GM
    NC = 6 * GM
    fb = pools["work"].tile([128, NB * 64], BF16, tag="fold", name="fold")
    fb2 = pools["work"].tile([128, NB * 32], BF16, tag="fold2", name="fold2")
    fb3 = pools["work"].tile([128, NB * 16], BF16, tag="fold3", name="fold3")
    ascr = pools["scr"].tile([128, 6 * 128], BF16, tag="ascr", name="ascr")

    fv = fb[:, 0:NB * L2].rearrange("p (k n) -> p k n", k=NB)
    fv2 = fb2[:, 0:NB * L4].rearrange("p (k n) -> p k n", k=NB)
    fv3 = fb3[:, 0:NB * L8].rearrange("p (k n) -> p k n", k=NB)
    pv = prod[:, 0:NH * L].rearrange("p (k n) -> p k n", k=NH)
    iv = in_sb[:, off:off + NC * L].rearrange("p (k n) -> p k n", k=NC)

    # fold1: H blocks [0,NH) from prod, c blocks [NH,NB) from input;
    # first bH/bC blocks on Pool, rest on DVE
    for dst0, srcv, nblk, npool in ((0, pv, NH, bH), (NH, iv, NC, bC)):
        for eng, k0, k1 in ((G, 0, npool), (V, npool, nblk)):
            if k0 >= k1:
                continue
            eng.tensor_tensor(
                out=fv[:, dst0 + k0:dst0 + k1, :],
                in0=srcv[:, k0:k1, 0:L2],
                in1=srcv[:, k0:k1, L2:2 * L2],
                op=Alu.add)

    # extra fold levels while profitable (halving pays iff width/2 >= 4)
    last = fv
    width = L2
    for nxt in (fv2, fv3):
        if width // 2 < 4:
            break
        w2 = width // 2
        V.tensor_tensor(out=nxt[:, :, 0:w2], in0=last[:, :, 0:w2],
                        in1=last[:, :, w2:width], op=Alu.add)
        last, width = nxt, w2
    st3 = st["all"][:, :, :]
    outH = _ap(st3, 15 * GM * m, [[15, GM], [1, 9]])
    V.tensor_reduce(out=outH, in_=last[:, 0:NH, 0:width], axis=AX.X,
                    op=Alu.add)
    outC = _ap(st3, 15 * GM * m + 9, [[15, GM], [1, 6]])
    V.tensor_reduce(out=outC, in_=last[:, NH:NB, 0:width], axis=AX.X,
                    op=Alu.add)

    # sppqq per sub-tile: ACT square with accumulate over [p, 6L]
    for g in range(GM):
        p0 = off + 6 * L * g
        t = GM * m + g
        A.activation(out=ascr[:, 0:6 * L], in_=in_sb[:, p0:p0 + 6 * L],
                     func=Act.Square,
                     accum_out=st["ss"][:, t:t + 1])


class P2:
    """Emit elementwise phase-2 ops on [128, C] column tiles."""

    def __init__(self, tc, pool, c0, c1, chunk, dma_out=None):
        self.nc = tc.nc
        self.pool = pool
        self.c0, self.c1 = c0, c1
        self.C = c1 - c0
        self.chunk = chunk
        self.ctr = 0
        self.dma_out = dma_out

    def mk(self, name=None):
        self.ctr += 1
        tag = f"c{self.chunk}_" + (name or f"t{self.ctr}")
        return self.pool.tile([128, self.C], F32, tag=tag, name=tag)

    def tt(self, a, b, op, eng=None, out=None):
        dst = out if out is not None else self.mk()
        (eng or self.nc.vector).tensor_tensor(out=dst, in0=a, in1=b, op=op)
        return dst

    def mul(self, a, b, eng=None, out=None):
        return self.tt(a, b, Alu.mult, eng, out)

    def add(self, a, b, eng=None, out=None):
        return self.tt(a, b, Alu.add, eng, out)

    def sub(self, a, b, eng=None, out=None):
        return self.tt(a, b, Alu.subtract, eng, out)

    def ts(self, a, s1, op0, s2=None, op1=Alu.bypass, eng=None, out=None):
        dst = out if out is not None else self.mk()
        (eng or self.nc.vector).tensor_scalar(
            out=dst, in0=a, scalar1=s1, scalar2=s2, op0=op0, op1=op1)
        return dst

    def stt(self, a, s, b, op0, op1, eng=None, out=None):
        """(a op0 s) op1 b in one instruction."""
        dst = out if out is not None else self.mk()
        (eng or self.nc.vector).scalar_tensor_tensor(
            out=dst, in0=a, scalar=s, in1=b, op0=op0, op1=op1)
        return dst

    def recip(self, a, out=None):
        dst = out if out is not None else self.mk()
        self.nc.vector.reciprocal(out=dst, in_=a)
        return dst

    def act(self, a, func, bias=0.0, scale=1.0, out=None):
        dst = out if out is not None else self.mk()
        self.nc.scalar.activation(out=dst, in_=a, func=func, bias=bias,
                                  scale=scale)
        return dst


def _phase2(tc, p2, st, cst, loss_out):
    """Per-sample Kabsch loss from stats, columns [c0, c1) (c = sub-tile).

    t = lam + 4*wx(lam)/p'(lam); lam = s1+s2+d*s3 via Cardano on K = Hc^T Hc;
    p'(lam) = 8(s2+d*s3)(s1+d*s3)(s1+s2); wx(lam) = -|a|^2 lam^2 + Wb lam + Wc
    is the adjugate-row-0 dot product, coefficients lam-free (computed early,
    off the critical path).
    """
    nc = tc.nc
    V, G, A = nc.vector, nc.gpsimd, nc.scalar
    c0, C = p2.c0, p2.C

    St = st["all"][:, :, :]        # [p, 32, 15]
    H9 = _ap(St, 15 * c0, [[15, C], [3, 3], [1, 3]])     # [p, c, i, j]
    sp_b = _ap(St, 15 * c0 + 9, [[15, C], [1, 3], [0, 3]])
    ss = st["ss"][:, c0:c0 + C]
    invn = cst[:, c0:c0 + C]
    invn3 = cst[:, N_TILES + c0:N_TILES + c0 + C]

    def wide(name, k):
        tag = f"c{p2.chunk}_{name}"
        return p2.pool.tile([128, C * k], F32, tag=tag, name=tag)

    # spqn = spq * invn (6-wide); corr = sum(spq*spqn); ppqqc = ss - corr
    spq6 = _ap(St, 15 * c0 + 9, [[15, C], [1, 6]])
    spqn6 = wide("spqn6", 6)
    spqn6_v = spqn6[:, :].rearrange("p (c k) -> p c k", k=6)
    inb6 = invn[:, :].unsqueeze(2).broadcast_to([128, C, 6])
    V.tensor_tensor(out=spqn6_v, in0=spq6, in1=inb6, op=Alu.mult)
    corrp = wide("corrp", 6)
    corrp_v = corrp[:, :].rearrange("p (c k) -> p c k", k=6)
    G.tensor_tensor(out=corrp_v, in0=spq6, in1=spqn6_v, op=Alu.mult)
    corr = p2.mk("corr")
    V.tensor_reduce(out=corr, in_=corrp_v, axis=AX.X, op=Alu.add)
    ppqqc = p2.sub(ss, corr, G)

    # centering: Hc[c, i, j] = H - sp_i * sqn_j
    mv = wide("mv", 9)
    mv_v = mv[:, :].rearrange("p (c i j) -> p c i j", i=3, j=3)
    sqn_b = bass.AP(tensor=spqn6_v.tensor, offset=spqn6_v.offset + 3,
                    ap=[spqn6_v.ap[0], [6, C], [0, 3], [1, 3]])
    V.tensor_tensor(out=mv_v, in0=sp_b, in1=sqn_b, op=Alu.mult)
    Hc = wide("Hc", 9)
    Hc_v = Hc[:, :].rearrange("p (c k) -> p c k", k=9)
    H9f = _ap(St, 15 * c0, [[15, C], [1, 9]])
    V.tensor_tensor(out=Hc_v, in0=H9f, in1=mv[:, :].rearrange(
        "p (c k) -> p c k", k=9), op=Alu.subtract)
    hc0 = Hc[:, :]
    h = {(i, j): _ap(hc0, 3 * i + j, [[9, C]]) for i in range(3)
         for j in range(3)}

    # K = Hc^T Hc: 3 products into one (c,a,b,i) tile, single reduce
    Kt = wide("Kt", 9)
    kp = wide("kp", 27)
    for aa in range(3):
        in0 = _ap(hc0, aa, [[9, C], [0, 3], [3, 3]])
        in1 = _ap(hc0, 0, [[9, C], [1, 3], [3, 3]])
        kp_v = _ap(kp[:, :], 9 * aa, [[27, C], [3, 3], [1, 3]])
        V.tensor_tensor(out=kp_v, in0=in0, in1=in1, op=Alu.mult)
    kp_flat = _ap(kp[:, :], 0, [[3, 9 * C], [1, 3]])
    V.tensor_reduce(out=Kt[:, :], in_=kp_flat, axis=AX.X, op=Alu.add)
    trK = p2.mk("trK")
    diag_v = _ap(Kt[:, :], 0, [[9, C], [4, 3]])
    V.tensor_reduce(out=trK, in_=diag_v, axis=AX.X, op=Alu.add)
    k2 = wide("k2", 9)
    V.tensor_tensor(out=k2[:, :], in0=Kt[:, :], in1=Kt[:, :], op=Alu.mult)
    trK2 = p2.mk("trK2")
    V.tensor_reduce(out=trK2, in_=k2[:, :].rearrange("p (c k) -> p c k", k=9),
                    axis=AX.X, op=Alu.add)

    # detH (of Hc) via 2x2 minors (Pool, off-spine)
    def minor2(pq, qq, rq_, sq_, eng=G):
        t1 = p2.mul(pq, qq, eng)
        t2 = p2.mul(rq_, sq_, eng)
        return p2.sub(t1, t2, eng)

    mm1 = minor2(h[(1, 1)], h[(2, 2)], h[(1, 2)], h[(2, 1)])
    mm2 = minor2(h[(1, 0)], h[(2, 2)], h[(1, 2)], h[(2, 0)])
    mm3 = minor2(h[(1, 0)], h[(2, 1)], h[(1, 1)], h[(2, 0)])
    dd1 = p2.mul(h[(0, 0)], mm1, G)
    dd2 = p2.mul(h[(0, 1)], mm2, G)
    dd3 = p2.mul(h[(0, 2)], mm3, G)
    detH = p2.add(p2.sub(dd1, dd2, G), dd3, G)
    sgn = p2.act(detH, Act.Sign)
    detK = p2.act(detH, Act.Square)

    # --- Cardano spine starts (DVE), W-coefficient work interleaved into
    # the spine's dependency-stall windows ---
    trKsq = p2.mul(trK, trK, V)
    p6 = p2.stt(trKsq, -1.0 / 3.0, trK2, Alu.mult, Alu.add, V)
    p6c = p2.ts(p6, 1e-12, Alu.max, eng=V)
    sqp = p2.act(p6c, Act.Sqrt, scale=1.0 / 6.0)         # sqrt(p)
    mmean = p2.ts(trK, 1.0 / 3.0, Alu.mult, eng=V)

    # [fill] Horn matrix entries of M = Hc^T, packed for one-shot squares
    npk1 = p2.pool.tile([128, 3 * C], F32, tag=f"c{p2.chunk}_npk1",
                        name="npk1")
    npk2 = p2.pool.tile([128, 3 * C], F32, tag=f"c{p2.chunk}_npk2",
                        name="npk2")
    n01 = p2.sub(h[(2, 1)], h[(1, 2)], G, out=npk1[:, 0:C])
    n02 = p2.sub(h[(0, 2)], h[(2, 0)], G, out=npk1[:, C:2 * C])
    n03 = p2.sub(h[(1, 0)], h[(0, 1)], G, out=npk1[:, 2 * C:3 * C])
    n23 = p2.add(h[(2, 1)], h[(1, 2)], V, out=npk2[:, 0:C])
    n13 = p2.add(h[(0, 2)], h[(2, 0)], V, out=npk2[:, C:2 * C])
    n12 = p2.add(h[(1, 0)], h[(0, 1)], V, out=npk2[:, 2 * C:3 * C])

    msq = p2.ts(trKsq, 1.0 / 9.0, Alu.mult, eng=V)
    m3c = p2.mul(msq, mmean, G)
    u = p2.stt(detK, 0.5, m3c, Alu.mult, Alu.add, V)     # m^3 + detK/2
    tdiff = p2.sub(trKsq, trK2, V)                       # 2*M2
    tm = p2.mul(tdiff, mmean, V)
    q = p2.stt(tm, -0.25, u, Alu.mult, Alu.add, V)
    p6sq = p2.mul(p6c, p6c, V)
    p3 = p2.mul(p6sq, p6c, V)
    q2 = p2.mul(q, q, V)
    pfloor = p2.ts(p3, 9.26e-11, Alu.mult, eng=V)
    diff = p2.stt(p3, 1.0 / 216.0, q2, Alu.mult, Alu.subtract, V)
    diffc = p2.tt(diff, pfloor, Alu.max, V)
    sqd = p2.act(diffc, Act.Sqrt)

    # [fill] squares of the packed entries + first W terms
    usq = p2.pool.tile([128, 3 * C], F32, tag=f"c{p2.chunk}_usq", name="usq")
    A.activation(out=usq[:, :], in_=npk1[:, :], func=Act.Square)
    u1, u2, u3 = usq[:, 0:C], usq[:, C:2 * C], usq[:, 2 * C:3 * C]
    wsq = p2.pool.tile([128, 3 * C], F32, tag=f"c{p2.chunk}_wsq", name="wsq")
    A.activation(out=wsq[:, :], in_=npk2[:, :], func=Act.Square)
    n23s, n13s, n12s = wsq[:, 0:C], wsq[:, C:2 * C], wsq[:, 2 * C:3 * C]
    tr3 = p2.add(p2.add(h[(0, 0)], h[(1, 1)], V), h[(2, 2)], V)
    n11 = p2.stt(h[(0, 0)], 2.0, tr3, Alu.mult, Alu.subtract, V)
    n22 = p2.stt(h[(1, 1)], 2.0, tr3, Alu.mult, Alu.subtract, V)
    n33 = p2.stt(h[(2, 2)], 2.0, tr3, Alu.mult, Alu.subtract, V)
    v1 = p2.mul(n01, n02, G)
    v2 = p2.mul(n01, n03, G)
    v3 = p2.mul(n02, n03, G)

    rq = p2.recip(sqd)
    ratio = p2.mul(q, rq, V)
    ratioc = p2.ts(ratio, 100.0, Alu.min, -100.0, Alu.max, V)
    at = p2.act(ratioc, Act.Arctan)

    # [fill] Wa, Wb
    Wa_n = p2.add(p2.add(u1, u2, V), u3, V)
    s1s = p2.add(n22, n33, G)
    s2s = p2.add(n11, n33, G)
    s3s = p2.add(n11, n22, G)
    b1 = p2.mul(u1, s1s, V)
    b2 = p2.mul(u2, s2s, V)
    b3 = p2.mul(u3, s3s, V)
    b4 = p2.mul(v1, n12, G)
    b5 = p2.mul(v2, n13, G)
    b6 = p2.mul(v3, n23, G)

    # packed Sin: [cos(phi) | sin(phi)] in one ACT op
    scp = wide("scp", 2)
    p2.ts(at, -1.0 / 3.0, Alu.mult, PI / 6.0 + PI / 2.0, Alu.add, V,
          out=scp[:, 0:C])
    p2.ts(at, -1.0 / 3.0, Alu.mult, PI / 6.0, Alu.add, V,
          out=scp[:, C:2 * C])
    sc = p2.pool.tile([128, 2 * C], F32, tag=f"c{p2.chunk}_sc", name="sc")
    A.activation(out=sc[:, :], in_=scp[:, :], func=Act.Sin)
    cphi = sc[:, 0:C]
    sphi = sc[:, C:2 * C]

    # [fill] Wb finish, Wc terms
    a123 = p2.add(p2.add(b1, b2, V), b3, V)
    c456 = p2.add(p2.add(b4, b5, G), b6, G)
    Wb = p2.stt(c456, -2.0, a123, Alu.mult, Alu.add, V)
    M1 = p2.sub(p2.mul(n22, n33, G), n23s, G)
    M2m = p2.sub(p2.mul(n11, n33, G), n13s, G)
    M3m = p2.sub(p2.mul(n11, n22, G), n12s, G)

    # eigenvalues via mp +/- sqrt(3)*ps; one packed Sqrt for all three
    pc = p2.mul(sqp, cphi, V)
    ps = p2.mul(sqp, sphi, V)
    lamp = p2.pool.tile([128, 3 * C], F32, tag=f"c{p2.chunk}_lamp",
                        name="lamp")
    p2.stt(pc, 2.0, mmean, Alu.mult, Alu.add, V, out=lamp[:, 0:C])
    mp = p2.sub(mmean, pc, V)
    s3p = p2.ts(ps, SQ3, Alu.mult, eng=V)
    lam2 = p2.add(mp, s3p, V)
    p2.ts(lam2, 0.0, Alu.max, eng=V, out=lamp[:, C:2 * C])
    lam3 = p2.sub(mp, s3p, V)
    p2.ts(lam3, 0.0, Alu.max, eng=V, out=lamp[:, 2 * C:3 * C])
    sgt = p2.pool.tile([128, 3 * C], F32, tag=f"c{p2.chunk}_sgt", name="sgt")
    A.activation(out=sgt[:, :], in_=lamp[:, :], func=Act.Sqrt)
    sg1 = sgt[:, 0:C]
    sg2 = sgt[:, C:2 * C]
    sg3 = sgt[:, 2 * C:3 * C]

    # [fill] Wc finish
    dd_ = p2.add(p2.add(p2.mul(u1, M1, G), p2.mul(u2, M2m, G), G),
                 p2.mul(u3, M3m, G), G)
    cc1 = p2.sub(p2.mul(n12, n33, V), p2.mul(n13, n23, V), V)
    cc2 = p2.sub(p2.mul(n12, n23, V), p2.mul(n13, n22, V), V)
    cc3 = p2.sub(p2.mul(n11, n23, V), p2.mul(n12, n13, V), V)
    ee = p2.add(p2.sub(p2.mul(v1, cc1, V), p2.mul(v2, cc2, V), V),
                p2.mul(v3, cc3, V), V)
    Wc = p2.stt(ee, 2.0, dd_, Alu.mult, Alu.subtract, V)   # 2*ee - dd

    s3d = p2.mul(sgn, sg3, V)
    t12 = p2.add(sg1, sg2, V)
    lam = p2.add(t12, s3d, V)                            # lambda_max of Horn
    pp1 = p2.add(sg2, s3d, G)
    pp2 = p2.add(sg1, s3d, G)
    ppr = p2.mul(pp1, pp2, G)
    ppr2 = p2.mul(ppr, t12, G)                           # p'(lam)/8
    pprc = p2.ts(ppr2, 1e-13, Alu.max, eng=V)
    rp = p2.recip(pprc)

    # wx = (-Wa_n*lam + Wb)*lam + Wc, then t and the loss
    wt1 = p2.mul(Wa_n, lam, V)
    wt2 = p2.sub(Wb, wt1, V)
    wt3 = p2.mul(wt2, lam, V)
    wx_v = p2.add(wt3, Wc, V)
    corr4 = p2.mul(wx_v, rp, V)
    t_unc = p2.stt(corr4, 0.5, lam, Alu.mult, Alu.add, V)  # lam + 4wx/p'
    ssum = p2.add(t12, sg3, V)
    tb = p2.tt(t_unc, ssum, Alu.min, V)
    ssn = p2.ts(ssum, -1.0, Alu.mult, eng=V)
    tcl = p2.tt(tb, ssn, Alu.max, V)
    li = p2.stt(tcl, -2.0, ppqqc, Alu.mult, Alu.add, V)
    p2.mul(li, invn3, V, out=loss_out)
    if p2.dma_out is not None:
        p2.dma_out(0, C)


def build_program(lmaxes, chunks=((0, 32),), n_dma=9, bH=28, bC=18,
                  wbufs=3, order="desc"):
    """lmaxes: per-merged-tile crop lengths (16 ints, multiples of 4)."""
    assert len(lmaxes) == N_MERGED
    tot = sum(6 * GM * L for L in lmaxes)
    offs = []
    o = 0
    for L in lmaxes:
        offs.append(o)
        o += 6 * GM * L

    nc = bass.Bass("TRN2", debug=False, enable_asserts=False,
                   target_bir_lowering=False)
    # extra activation-bias constants (only 0.0/1.0 pre-registered)
    for cval in (PI / 2.0,):
        cten = nc.alloc_sbuf_tensor(f"const-f32-{cval}", [128, 1], F32)
        nc.gpsimd.memset(cten.ap(), cval)
        nc.const_aps.aps[(F32, cval)] = cten.ap()
    nc.all_engine_barrier()
    pq = nc.dram_tensor("pq", [128, tot], BF16, kind="ExternalInput").ap()
    cstd = nc.dram_tensor("cst", [128, 2 * N_TILES], F32,
                          kind="ExternalInput").ap()
    loss = nc.dram_tensor("loss", [128, N_TILES], F32,
                          kind="ExternalOutput").ap()

    with tile.TileContext(nc) as tc:
        from contextlib import ExitStack
        with ExitStack() as ctx:
            pools = {
                "in": ctx.enter_context(tc.tile_pool(name="inp", bufs=1)),
                "work": ctx.enter_context(tc.tile_pool(name="work", bufs=wbufs)),
                "scr": ctx.enter_context(tc.tile_pool(name="scr", bufs=3)),
                "stats": ctx.enter_context(tc.tile_pool(name="stats", bufs=1)),
                "ph2": ctx.enter_context(tc.tile_pool(name="ph2", bufs=1)),
            }
            pools["bH"] = bH
            pools["bC"] = bC
            in_sb = pools["in"].tile([128, tot], BF16, tag="in", name="in")
            cst = pools["stats"].tile([128, 2 * N_TILES], F32, tag="cst",
                                      name="cst")
            st = {
                "all": pools["stats"].tile([128, N_TILES, 15], F32,
                                           tag="st_all", name="st_all"),
                "ss": pools["stats"].tile([128, N_TILES], F32,
                                          tag="st_ss", name="st_ss"),
            }
            loss_tile = pools["ph2"].tile([128, N_TILES], F32, tag="loss",
                                          name="loss")

            # input DMAs: small first chunk so compute starts early, then
            # n_dma-1 even chunks over the rest; cst after the first chunk
            bounds = [0, 1]
            rem = N_MERGED - 1
            for d in range(n_dma - 1):
                bounds.append(1 + ((d + 1) * rem) // (n_dma - 1))
            first = True
            for ma, mb in zip(bounds[:-1], bounds[1:]):
                if ma >= mb:
                    continue
                e0 = offs[ma]
                e1 = offs[mb - 1] + 6 * GM * lmaxes[mb - 1]
                if first:
                    # halve the first chunk so compute can start sooner
                    eh = e0 + 3 * GM * lmaxes[ma]
                    nc.sync.dma_start(out=in_sb[:, e0:eh], in_=pq[:, e0:eh])
                    nc.sync.dma_start(out=in_sb[:, eh:e1], in_=pq[:, eh:e1])
                    nc.sync.dma_start(out=cst[:, :], in_=cstd)
                    first = False
                else:
                    nc.sync.dma_start(out=in_sb[:, e0:e1], in_=pq[:, e0:e1])

            ci = 0
            prods = {}
            prods[0] = _emit_products(tc, pools, in_sb, 0, lmaxes[0], offs[0])
            for m in range(N_MERGED):
                if m + 1 < N_MERGED:
                    prods[m + 1] = _emit_products(
                        tc, pools, in_sb, m + 1, lmaxes[m + 1], offs[m + 1])
                _phase1_rest(tc, pools, in_sb, st, m, lmaxes[m], offs[m],
                             prods.pop(m))
                while ci < len(chunks) and GM * (m + 1) >= chunks[ci][1]:
                    a, b = chunks[ci]

                    def _dma_out(x0, x1, a=a):
                        nc.sync.dma_start(out=loss[:, a + x0:a + x1],
                                          in_=loss_tile[:, a + x0:a + x1])
                    p2 = P2(tc, pools["ph2"], a, b, ci, dma_out=_dma_out)
                    _phase2(tc, p2, st, cst, loss_tile[:, a:b])
                    ci += 1
    _legalize_single_wait(nc)
    return nc


_nc_cache = {}


def _get_program(lmaxes, chunks=((0, 32),), n_dma=9, bH=28, bC=18, wbufs=3,
                 order="desc"):
    key = (lmaxes, chunks, n_dma, bH, bC, wbufs, order)
    if key not in _nc_cache:
        _nc_cache[key] = build_program(lmaxes, chunks, n_dma, bH, bC, wbufs,
                                       order)
    return _nc_cache[key]


def _prep(pred_coord, true_coord, pad_mask, torder="desc"):
    """Host-side packing. Returns (lmaxes, in_maps)."""
    P = np.asarray(pred_coord, dtype=np.float32)
    Q = np.asarray(true_coord, dtype=np.float32)
    M = np.asarray(pad_mask)
    B = P.shape[0]
    assert B == B_FULL and P.shape[1] == N_SEQ

    lengths = (N_SEQ - M.sum(axis=1)).astype(np.int64)
    order = np.argsort(lengths, kind="stable")
    lsort = lengths[order]
    # merged tile m takes sorted block blk[m]; longest first so the early
    # DMA chunks carry the most compute
    if torder == "ilv":
        blk = []
        hi, lo = N_MERGED - 1, N_MERGED // 2 - 1
        for i in range(N_MERGED // 2):
            blk.append(hi - i)
            blk.append(lo - i)
        blk = tuple(blk)
    else:
        blk = tuple(range(N_MERGED - 1, -1, -1))
    bsz = 1024 * GM
    lmaxes = []
    for m in range(N_MERGED):
        L = int(lsort[bsz * (blk[m] + 1) - 1])
        L = max(8, (L + 7) & ~7)
        lmaxes.append(L)
    lmaxes = tuple(lmaxes)

    # zero padding, transpose to [B, 3, N], sort
    w = (np.arange(N_SEQ)[None, :] < lengths[:, None]).astype(np.float32)
    Pz = (P * w[:, :, None]).transpose(0, 2, 1)[order]   # [B, 3, N]
    Qz = (Q * w[:, :, None]).transpose(0, 2, 1)[order]
    Pb = Pz.astype(ml_dtypes.bfloat16)
    Qb = Qz.astype(ml_dtypes.bfloat16)

    tot = sum(6 * GM * L for L in lmaxes)
    in_maps = []
    linv = (1.0 / lsort.astype(np.float64)).astype(np.float32)
    for c in range(N_CORES):
        buf = np.zeros((128, tot), dtype=ml_dtypes.bfloat16)
        o = 0
        for m in range(N_MERGED):
            L = lmaxes[m]
            bm = blk[m]
            gsel = np.arange(bsz * bm + c, bsz * (bm + 1), 8)  # 128*GM sorted
            Pm = Pb[gsel][:, :, :L]      # [128*GM, 3, L]
            Qm = Qb[gsel][:, :, :L]
            for g in range(GM):
                sl = slice(128 * g, 128 * (g + 1))
                buf[:, o:o + 3 * L] = Pm[sl].reshape(128, 3 * L)
                buf[:, o + 3 * L:o + 6 * L] = Qm[sl].reshape(128, 3 * L)
                o += 6 * L
        # constants: invn (32 cols), invn/3 (32 cols); col t, partition p
        # -> sorted index (t*128+p)*8 + c
        idx = (np.arange(B_CORE) * 8 + c)
        nin = linv[idx].reshape(N_TILES, 128).T          # [128, 32]
        # column t = sub-tile GM*m+g holds sorted sub-block GM*blk[m]+g
        perm = [GM * blk[t // GM] + (t % GM) for t in range(N_TILES)]
        nin = nin[:, perm]
        cstv = np.concatenate([nin, nin / 3.0], axis=1).astype(np.float32)
        in_maps.append({"pq": buf, "cst": np.ascontiguousarray(cstv)})
    return lmaxes, in_maps


def kernel(pred_coord, true_coord, pad_mask):
    lmaxes, in_maps = _prep(pred_coord, true_coord, pad_mask)
    nc = _get_program(lmaxes)
    trace = bool(int(os.environ.get("KERNEL_TRACE", "0")))
    res = run_bass_kernel_spmd(nc, in_maps, core_ids=list(range(N_CORES)),
                               trace=trace)
    if trace and res.exec_time_ns is not None:
        print(f"HW exec time: {res.exec_time_ns} ns")
        kernel.last_exec_time_ns = res.exec_time_ns
    total = 0.0
    for r in res.results:
        total += r["loss"].astype(np.float64).sum()
    return np.float32(total / B_FULL)


kernel.last_exec_time_ns = None


# revision 4
# speedup vs baseline: 1.0151x; 1.0058x over previous
"""Trainium2 Bass kernel v2: batched Kabsch-aligned masked MSE.

Math: per-sample loss = (|Pc|^2+|Qc|^2 - 2 t)/(3n) with t = s1+s2+sign(detH)*s3,
s_i = singular values of the 3x3 cross-covariance H = Pc^T Qc.  s_i^2 are the
eigenvalues of K = H^T H, found in closed form (Cardano / trigonometric method
using Arctan+Sin on the ACT engine).  No eigenvector needed.

Layout: samples sorted by valid length, striped over 8 cores; on-core 32 tiles
of 128 samples (samples on partitions), pairs of tiles merged (shared length
crop L).  Inputs are bf16, zero-padded on the host, shipped pre-transposed
as [P_A | Q_A | P_B | Q_B] per partition row, in a handful of large
contiguous DMAs.  Phase 1 computes per-sample sums (H, sp, sq, sppqq) with
DVE bf16 2x products + Pool folds + DVE reduces + ACT square-accum.  Phase 2
solves the 3x3 eigenproblem elementwise on [128, C] stat tiles.
"""

import os
import numpy as np
import ml_dtypes

import bass_rust
import concourse.bass as bass
import concourse.tile as tile
from concourse import mybir
from concourse.bass_utils import run_bass_kernel_spmd

F32 = mybir.dt.float32
BF16 = mybir.dt.bfloat16
Alu = mybir.AluOpType
Act = mybir.ActivationFunctionType
AX = mybir.AxisListType

N_CORES = 8
B_FULL = 32768
N_SEQ = 128
B_CORE = B_FULL // N_CORES      # 4096
N_TILES = B_CORE // 128         # 32 sub-tiles
GM = 4                          # sub-tiles per merged tile
N_MERGED = N_TILES // GM        # merged tiles
SQ3 = 1.7320508075688772
PI = 3.141592653589793


def _legalize_single_wait(nc):
    """Split multi-wait instructions into chains of single-wait Drains
    (deployed walrus build allows only one sync-wait per instruction)."""
    moved = 0
    for fn in nc.m.functions:
        for blk in fn.blocks:
            insts = blk.instructions
            new_list = []
            for ins in insts:
                si = ins.sync_info
                ow = list(si.on_wait) if si is not None and si.on_wait else []
                if len(ow) > 1:
                    for w in ow[:-1]:
                        d = mybir.InstDrain(name=f"I-sw{moved}", ins=[],
                                            outs=[], bass_is_fusable=False)
                        d.engine = ins.engine
                        d.sync_info = bass_rust.SyncInfo(on_wait=[w],
                                                         on_update=[])
                        new_list.append(d)
                        moved += 1
                    si.on_wait = [ow[-1]]
                new_list.append(ins)
            blk.instructions[:] = new_list
    return moved


def _ap(base, extra_offset, dims):
    """Manual AP: keep base's partition dim, replace free dims."""
    return bass.AP(tensor=base.tensor, offset=base.offset + extra_offset,
                   ap=[base.ap[0]] + [list(d) for d in dims])


def _emit_products(tc, pools, in_sb, m, L, off):
    """Products for merged tile m -> bf16 tile [p, 18, L], g-major blocks."""
    nc = tc.nc
    V = nc.vector
    prod = pools["work"].tile([128, 9 * GM * 128], BF16, tag="prod",
                              name="prod")
    for g in range(GM):
        p0 = off + 6 * L * g
        Pv = (in_sb[:, p0:p0 + 3 * L]
              .rearrange("p (i n) -> p i n", i=3)
              .unsqueeze(2).broadcast_to([128, 3, 3, L]))
        Qv = (in_sb[:, p0 + 3 * L:p0 + 6 * L]
              .rearrange("p (j n) -> p j n", j=3)
              .unsqueeze(1).broadcast_to([128, 3, 3, L]))
        out = prod[:, 9 * L * g:9 * L * (g + 1)].rearrange(
            "p (i j n) -> p i j n", i=3, j=3)
        V.tensor_tensor(out=out, in0=Pv, in1=Qv, op=Alu.mult)
    return prod


def _phase1_rest(tc, pools, in_sb, st, m, L, off, prod):
    """Folds + reduce + sppqq for merged tile m.

    Combined fold buffer blocks (30 x L2): [H_A(9) H_B(9) c_A(6) c_B(6)];
    three fold levels, then two TRs write st['all'][:, 2m:2m+2, :]
    (per sub-tile 15 = H(9), sp(3), sq(3)).
    """
    nc = tc.nc
    V, G, A = nc.vector, nc.gpsimd, nc.scalar
    L2, L4, L8 = L // 2, L // 4, L // 8
    bH = pools.get("bH", 28)    # of 9*GM H-fold blocks on Pool
    bC = pools.get("bC", 18)    # of 6*GM c-fold blocks on Pool
    if L <= pools.get("poolmin", 0):
        bH = bC = 0             # short tiles: avoid cross-engine latency

    NB = 15 * GM
    NH = 9 * GM
    NC = 6 * GM
    fb = pools["work"].tile([128, NB * 64], BF16, tag="fold", name="fold")
    fb2 = pools["work"].tile([128, NB * 32], BF16, tag="fold2", name="fold2")
    fb3 = pools["work"].tile([128, NB * 16], BF16, tag="fold3", name="fold3")
    ascr = pools["scr"].tile([128, 6 * 128], BF16, tag="ascr", name="ascr")

    fv = fb[:, 0:NB * L2].rearrange("p (k n) -> p k n", k=NB)
    fv2 = fb2[:, 0:NB * L4].rearrange("p (k n) -> p k n", k=NB)
    fv3 = fb3[:, 0:NB * L8].rearrange("p (k n) -> p k n", k=NB)
    pv = prod[:, 0:NH * L].rearrange("p (k n) -> p k n", k=NH)
    iv = in_sb[:, off:off + NC * L].rearrange("p (k n) -> p k n", k=NC)

    # fold1: H blocks [0,NH) from prod, c blocks [NH,NB) from input;
    # first bH/bC blocks on Pool, rest on DVE
    for dst0, srcv, nblk, npool in ((0, pv, NH, bH), (NH, iv, NC, bC)):
        for eng, k0, k1 in ((G, 0, npool), (V, npool, nblk)):
            if k0 >= k1:
                continue
            eng.tensor_tensor(
                out=fv[:, dst0 + k0:dst0 + k1, :],
                in0=srcv[:, k0:k1, 0:L2],
                in1=srcv[:, k0:k1, L2:2 * L2],
                op=Alu.add)

    # extra fold levels while profitable (halving pays iff width/2 >= 4)
    bF2 = pools.get("bF2", 0)   # fold2 blocks on Pool
    last = fv
    width = L2
    for lvl, nxt in enumerate((fv2, fv3)):
        if width // 2 < 4:
            break
        w2 = width // 2
        npool = bF2 if lvl == 0 else 0
        for eng, k0, k1 in ((G, 0, npool), (V, npool, NB)):
            if k0 >= k1:
                continue
            eng.tensor_tensor(out=nxt[:, k0:k1, 0:w2],
                              in0=last[:, k0:k1, 0:w2],
                              in1=last[:, k0:k1, w2:width], op=Alu.add)
        last, width = nxt, w2
    st3 = st["all"][:, :, :]
    outH = _ap(st3, 15 * GM * m, [[15, GM], [1, 9]])
    V.tensor_reduce(out=outH, in_=last[:, 0:NH, 0:width], axis=AX.X,
                    op=Alu.add)
    outC = _ap(st3, 15 * GM * m + 9, [[15, GM], [1, 6]])
    V.tensor_reduce(out=outC, in_=last[:, NH:NB, 0:width], axis=AX.X,
                    op=Alu.add)

    # sppqq per sub-tile: ACT square with accumulate over [p, 6L]
    for g in range(GM):
        p0 = off + 6 * L * g
        t = GM * m + g
        A.activation(out=ascr[:, 0:6 * L], in_=in_sb[:, p0:p0 + 6 * L],
                     func=Act.Square,
                     accum_out=st["ss"][:, t:t + 1])


class P2:
    """Emit elementwise phase-2 ops on [128, C] column tiles."""

    def __init__(self, tc, pool, c0, c1, chunk, dma_out=None):
        self.nc = tc.nc
        self.pool = pool
        self.c0, self.c1 = c0, c1
        self.C = c1 - c0
        self.chunk = chunk
        self.ctr = 0
        self.dma_out = dma_out

    def mk(self, name=None):
        self.ctr += 1
        tag = f"c{self.chunk}_" + (name or f"t{self.ctr}")
        return self.pool.tile([128, self.C], F32, tag=tag, name=tag)

    def tt(self, a, b, op, eng=None, out=None):
        dst = out if out is not None else self.mk()
        (eng or self.nc.vector).tensor_tensor(out=dst, in0=a, in1=b, op=op)
        return dst

    def mul(self, a, b, eng=None, out=None):
        return self.tt(a, b, Alu.mult, eng, out)

    def add(self, a, b, eng=None, out=None):
        return self.tt(a, b, Alu.add, eng, out)

    def sub(self, a, b, eng=None, out=None):
        return self.tt(a, b, Alu.subtract, eng, out)

    def ts(self, a, s1, op0, s2=None, op1=Alu.bypass, eng=None, out=None):
        dst = out if out is not None else self.mk()
        (eng or self.nc.vector).tensor_scalar(
            out=dst, in0=a, scalar1=s1, scalar2=s2, op0=op0, op1=op1)
        return dst

    def stt(self, a, s, b, op0, op1, eng=None, out=None):
        """(a op0 s) op1 b in one instruction."""
        dst = out if out is not None else self.mk()
        (eng or self.nc.vector).scalar_tensor_tensor(
            out=dst, in0=a, scalar=s, in1=b, op0=op0, op1=op1)
        return dst

    def recip(self, a, out=None):
        dst = out if out is not None else self.mk()
        self.nc.vector.reciprocal(out=dst, in_=a)
        return dst

    def act(self, a, func, bias=0.0, scale=1.0, out=None):
        dst = out if out is not None else self.mk()
        self.nc.scalar.activation(out=dst, in_=a, func=func, bias=bias,
                                  scale=scale)
        return dst


def _phase2(tc, p2, st, cst, loss_out):
    """Per-sample Kabsch loss from stats, columns [c0, c1) (c = sub-tile).

    t = lam + 4*wx(lam)/p'(lam); lam = s1+s2+d*s3 via Cardano on K = Hc^T Hc;
    p'(lam) = 8(s2+d*s3)(s1+d*s3)(s1+s2); wx(lam) = -|a|^2 lam^2 + Wb lam + Wc
    is the adjugate-row-0 dot product, coefficients lam-free (computed early,
    off the critical path).
    """
    nc = tc.nc
    V, G, A = nc.vector, nc.gpsimd, nc.scalar
    c0, C = p2.c0, p2.C

    St = st["all"][:, :, :]        # [p, 32, 15]
    H9 = _ap(St, 15 * c0, [[15, C], [3, 3], [1, 3]])     # [p, c, i, j]
    sp_b = _ap(St, 15 * c0 + 9, [[15, C], [1, 3], [0, 3]])
    ss = st["ss"][:, c0:c0 + C]
    invn = cst[:, c0:c0 + C]
    invn3 = cst[:, N_TILES + c0:N_TILES + c0 + C]

    def wide(name, k):
        tag = f"c{p2.chunk}_{name}"
        return p2.pool.tile([128, C * k], F32, tag=tag, name=tag)

    # spqn = spq * invn (6-wide); corr = sum(spq*spqn); ppqqc = ss - corr
    spq6 = _ap(St, 15 * c0 + 9, [[15, C], [1, 6]])
    spqn6 = wide("spqn6", 6)
    spqn6_v = spqn6[:, :].rearrange("p (c k) -> p c k", k=6)
    inb6 = invn[:, :].unsqueeze(2).broadcast_to([128, C, 6])
    V.tensor_tensor(out=spqn6_v, in0=spq6, in1=inb6, op=Alu.mult)
    corrp = wide("corrp", 6)
    corrp_v = corrp[:, :].rearrange("p (c k) -> p c k", k=6)
    G.tensor_tensor(out=corrp_v, in0=spq6, in1=spqn6_v, op=Alu.mult)
    corr = p2.mk("corr")
    V.tensor_reduce(out=corr, in_=corrp_v, axis=AX.X, op=Alu.add)
    ppqqc = p2.sub(ss, corr, G)

    # centering: Hc[c, i, j] = H - sp_i * sqn_j
    mv = wide("mv", 9)
    mv_v = mv[:, :].rearrange("p (c i j) -> p c i j", i=3, j=3)
    sqn_b = bass.AP(tensor=spqn6_v.tensor, offset=spqn6_v.offset + 3,
                    ap=[spqn6_v.ap[0], [6, C], [0, 3], [1, 3]])
    V.tensor_tensor(out=mv_v, in0=sp_b, in1=sqn_b, op=Alu.mult)
    Hc = wide("Hc", 9)
    Hc_v = Hc[:, :].rearrange("p (c k) -> p c k", k=9)
    H9f = _ap(St, 15 * c0, [[15, C], [1, 9]])
    V.tensor_tensor(out=Hc_v, in0=H9f, in1=mv[:, :].rearrange(
        "p (c k) -> p c k", k=9), op=Alu.subtract)
    hc0 = Hc[:, :]
    h = {(i, j): _ap(hc0, 3 * i + j, [[9, C]]) for i in range(3)
         for j in range(3)}

    # K = Hc^T Hc: 3 products into one (c,a,b,i) tile, single reduce
    Kt = wide("Kt", 9)
    kp = wide("kp", 27)
    for aa in range(3):
        in0 = _ap(hc0, aa, [[9, C], [0, 3], [3, 3]])
        in1 = _ap(hc0, 0, [[9, C], [1, 3], [3, 3]])
        kp_v = _ap(kp[:, :], 9 * aa, [[27, C], [3, 3], [1, 3]])
        V.tensor_tensor(out=kp_v, in0=in0, in1=in1, op=Alu.mult)
    kp_flat = _ap(kp[:, :], 0, [[3, 9 * C], [1, 3]])
    V.tensor_reduce(out=Kt[:, :], in_=kp_flat, axis=AX.X, op=Alu.add)
    trK = p2.mk("trK")
    diag_v = _ap(Kt[:, :], 0, [[9, C], [4, 3]])
    V.tensor_reduce(out=trK, in_=diag_v, axis=AX.X, op=Alu.add)
    k2 = wide("k2", 9)
    V.tensor_tensor(out=k2[:, :], in0=Kt[:, :], in1=Kt[:, :], op=Alu.mult)
    trK2 = p2.mk("trK2")
    V.tensor_reduce(out=trK2, in_=k2[:, :].rearrange("p (c k) -> p c k", k=9),
                    axis=AX.X, op=Alu.add)

    # detH (of Hc) via 2x2 minors (Pool, off-spine)
    def minor2(pq, qq, rq_, sq_, eng=G):
        t1 = p2.mul(pq, qq, eng)
        t2 = p2.mul(rq_, sq_, eng)
        return p2.sub(t1, t2, eng)

    mm1 = minor2(h[(1, 1)], h[(2, 2)], h[(1, 2)], h[(2, 1)])
    mm2 = minor2(h[(1, 0)], h[(2, 2)], h[(1, 2)], h[(2, 0)])
    mm3 = minor2(h[(1, 0)], h[(2, 1)], h[(1, 1)], h[(2, 0)])
    dd1 = p2.mul(h[(0, 0)], mm1, G)
    dd2 = p2.mul(h[(0, 1)], mm2, G)
    dd3 = p2.mul(h[(0, 2)], mm3, G)
    detH = p2.add(p2.sub(dd1, dd2, G), dd3, G)
    sgn = p2.act(detH, Act.Sign)
    detK = p2.act(detH, Act.Square)

    # --- Cardano spine starts (DVE), W-coefficient work interleaved into
    # the spine's dependency-stall windows ---
    trKsq = p2.mul(trK, trK, V)
    p6 = p2.stt(trKsq, -1.0 / 3.0, trK2, Alu.mult, Alu.add, V)
    p6c = p2.ts(p6, 1e-12, Alu.max, eng=V)
    sqp = p2.act(p6c, Act.Sqrt, scale=1.0 / 6.0)         # sqrt(p)
    mmean = p2.ts(trK, 1.0 / 3.0, Alu.mult, eng=V)

    # [fill] Horn matrix entries of M = Hc^T, packed for one-shot squares
    npk1 = p2.pool.tile([128, 3 * C], F32, tag=f"c{p2.chunk}_npk1",
                        name="npk1")
    npk2 = p2.pool.tile([128, 3 * C], F32, tag=f"c{p2.chunk}_npk2",
                        name="npk2")
    n01 = p2.sub(h[(2, 1)], h[(1, 2)], G, out=npk1[:, 0:C])
    n02 = p2.sub(h[(0, 2)], h[(2, 0)], G, out=npk1[:, C:2 * C])
    n03 = p2.sub(h[(1, 0)], h[(0, 1)], G, out=npk1[:, 2 * C:3 * C])
    n23 = p2.add(h[(2, 1)], h[(1, 2)], V, out=npk2[:, 0:C])
    n13 = p2.add(h[(0, 2)], h[(2, 0)], V, out=npk2[:, C:2 * C])
    n12 = p2.add(h[(1, 0)], h[(0, 1)], V, out=npk2[:, 2 * C:3 * C])

    msq = p2.ts(trKsq, 1.0 / 9.0, Alu.mult, eng=V)
    m3c = p2.mul(msq, mmean, G)
    u = p2.stt(detK, 0.5, m3c, Alu.mult, Alu.add, V)     # m^3 + detK/2
    tdiff = p2.sub(trKsq, trK2, V)                       # 2*M2
    tm = p2.mul(tdiff, mmean, V)
    q = p2.stt(tm, -0.25, u, Alu.mult, Alu.add, V)
    p6sq = p2.mul(p6c, p6c, V)
    p3 = p2.mul(p6sq, p6c, V)
    q2 = p2.mul(q, q, V)
    pfloor = p2.ts(p3, 9.26e-11, Alu.mult, 1e-38, Alu.max, eng=V)
    diff = p2.stt(p3, 1.0 / 216.0, q2, Alu.mult, Alu.subtract, V)
    diffc = p2.tt(diff, pfloor, Alu.max, V)
    sqd = p2.act(diffc, Act.Sqrt)

    # [fill] squares of the packed entries + first W terms
    usq = p2.pool.tile([128, 3 * C], F32, tag=f"c{p2.chunk}_usq", name="usq")
    A.activation(out=usq[:, :], in_=npk1[:, :], func=Act.Square)
    u1, u2, u3 = usq[:, 0:C], usq[:, C:2 * C], usq[:, 2 * C:3 * C]
    wsq = p2.pool.tile([128, 3 * C], F32, tag=f"c{p2.chunk}_wsq", name="wsq")
    A.activation(out=wsq[:, :], in_=npk2[:, :], func=Act.Square)
    n23s, n13s, n12s = wsq[:, 0:C], wsq[:, C:2 * C], wsq[:, 2 * C:3 * C]
    tr3 = p2.add(p2.add(h[(0, 0)], h[(1, 1)], V), h[(2, 2)], V)
    n11 = p2.stt(h[(0, 0)], 2.0, tr3, Alu.mult, Alu.subtract, V)
    n22 = p2.stt(h[(1, 1)], 2.0, tr3, Alu.mult, Alu.subtract, V)
    n33 = p2.stt(h[(2, 2)], 2.0, tr3, Alu.mult, Alu.subtract, V)
    v1 = p2.mul(n01, n02, G)
    v2 = p2.mul(n01, n03, G)
    v3 = p2.mul(n02, n03, G)

    rq = p2.recip(sqd)
    ratio = p2.mul(q, rq, V)
    ratioc = p2.ts(ratio, 100.0, Alu.min, -100.0, Alu.max, V)
    at = p2.act(ratioc, Act.Arctan)

    # [fill] Wa, Wb
    Wa_n = p2.add(p2.add(u1, u2, V), u3, V)
    s1s = p2.add(n22, n33, G)
    s2s = p2.add(n11, n33, G)
    s3s = p2.add(n11, n22, G)
    b1 = p2.mul(u1, s1s, V)
    b2 = p2.mul(u2, s2s, V)
    b3 = p2.mul(u3, s3s, V)
    b4 = p2.mul(v1, n12, G)
    b5 = p2.mul(v2, n13, G)
    b6 = p2.mul(v3, n23, G)

    # cos/sin of phi straight from `at` on ACT (func(scale*x+bias)):
    # stays on the ACT queue, no DVE round-trip
    cphi = p2.act(at, Act.Sin, bias=PI / 6.0 + PI / 2.0, scale=-1.0 / 3.0)
    sphi = p2.act(at, Act.Sin, bias=PI / 6.0, scale=-1.0 / 3.0)

    # [fill] Wb finish, Wc terms
    a123 = p2.add(p2.add(b1, b2, V), b3, V)
    c456 = p2.add(p2.add(b4, b5, G), b6, G)
    Wb = p2.stt(c456, -2.0, a123, Alu.mult, Alu.add, V)
    M1 = p2.sub(p2.mul(n22, n33, G), n23s, G)
    M2m = p2.sub(p2.mul(n11, n33, G), n13s, G)
    M3m = p2.sub(p2.mul(n11, n22, G), n12s, G)

    # eigenvalues via mp +/- sqrt(3)*ps; one packed Sqrt for all three
    pc = p2.mul(sqp, cphi, V)
    ps = p2.mul(sqp, sphi, V)
    lamp = p2.pool.tile([128, 3 * C], F32, tag=f"c{p2.chunk}_lamp",
                        name="lamp")
    p2.stt(pc, 2.0, mmean, Alu.mult, Alu.add, V, out=lamp[:, 0:C])
    mp = p2.sub(mmean, pc, V)
    s3p = p2.ts(ps, SQ3, Alu.mult, eng=V)
    lam2 = p2.add(mp, s3p, V)
    p2.ts(lam2, 0.0, Alu.max, eng=V, out=lamp[:, C:2 * C])
    lam3 = p2.sub(mp, s3p, V)
    p2.ts(lam3, 0.0, Alu.max, eng=V, out=lamp[:, 2 * C:3 * C])
    sgt = p2.pool.tile([128, 3 * C], F32, tag=f"c{p2.chunk}_sgt", name="sgt")
    A.activation(out=sgt[:, :], in_=lamp[:, :], func=Act.Sqrt)
    sg1 = sgt[:, 0:C]
    sg2 = sgt[:, C:2 * C]
    sg3 = sgt[:, 2 * C:3 * C]

    # [fill] Wc finish
    dd_ = p2.add(p2.add(p2.mul(u1, M1, G), p2.mul(u2, M2m, G), G),
                 p2.mul(u3, M3m, G), G)
    cc1 = p2.sub(p2.mul(n12, n33, V), p2.mul(n13, n23, V), V)
    cc2 = p2.sub(p2.mul(n12, n23, V), p2.mul(n13, n22, V), V)
    cc3 = p2.sub(p2.mul(n11, n23, V), p2.mul(n12, n13, V), V)
    ee = p2.add(p2.sub(p2.mul(v1, cc1, V), p2.mul(v2, cc2, V), V),
                p2.mul(v3, cc3, V), V)
    Wc = p2.stt(ee, 2.0, dd_, Alu.mult, Alu.subtract, V)   # 2*ee - dd

    s3d = p2.mul(sgn, sg3, V)
    t12 = p2.add(sg1, sg2, V)
    lam = p2.add(t12, s3d, V)                            # lambda_max of Horn
    pp1 = p2.add(sg2, s3d, V)
    pp2 = p2.add(sg1, s3d, V)
    ppr = p2.mul(pp1, pp2, V)
    ppr2 = p2.mul(ppr, t12, V)                           # p'(lam)/8
    pprc = p2.ts(ppr2, 1e-13, Alu.max, eng=V)
    rp = p2.recip(pprc)

    # wx = (-Wa_n*lam + Wb)*lam + Wc, then t and the loss
    wt1 = p2.mul(Wa_n, lam, V)
    wt2 = p2.sub(Wb, wt1, V)
    wt3 = p2.mul(wt2, lam, V)
    wx_v = p2.add(wt3, Wc, V)
    corr4 = p2.mul(wx_v, rp, V)
    t_unc = p2.stt(corr4, 0.5, lam, Alu.mult, Alu.add, V)  # lam + 4wx/p'
    ssum = p2.add(t12, sg3, V)
    tb = p2.tt(t_unc, ssum, Alu.min, V)
    ssn = p2.ts(ssum, -1.0, Alu.mult, eng=V)
    tcl = p2.tt(tb, ssn, Alu.max, V)
    li = p2.stt(tcl, -2.0, ppqqc, Alu.mult, Alu.add, V)
    p2.mul(li, invn3, V, out=loss_out)
    if p2.dma_out is not None:
        p2.dma_out(0, C)


def build_program(lmaxes, chunks=((0, 32),), n_dma=9, bH=28, bC=18,
                  wbufs=3, order="desc", bF2=0):
    """lmaxes: per-merged-tile crop lengths (16 ints, multiples of 4)."""
    assert len(lmaxes) == N_MERGED
    tot = sum(6 * GM * L for L in lmaxes)
    offs = []
    o = 0
    for L in lmaxes:
        offs.append(o)
        o += 6 * GM * L

    nc = bass.Bass("TRN2", debug=False, enable_asserts=False,
                   target_bir_lowering=False)
    # extra activation-bias constants (only 0.0/1.0 pre-registered)
    for cval in (PI / 2.0, PI / 6.0 + PI / 2.0, PI / 6.0):
        cten = nc.alloc_sbuf_tensor(f"const-f32-{cval}", [128, 1], F32)
        nc.gpsimd.memset(cten.ap(), cval)
        nc.const_aps.aps[(F32, cval)] = cten.ap()
    nc.all_engine_barrier()
    pq = nc.dram_tensor("pq", [128, tot], BF16, kind="ExternalInput").ap()
    cstd = nc.dram_tensor("cst", [128, 2 * N_TILES], F32,
                          kind="ExternalInput").ap()
    loss = nc.dram_tensor("loss", [128, N_TILES], F32,
                          kind="ExternalOutput").ap()

    with tile.TileContext(nc) as tc:
        from contextlib import ExitStack
        with ExitStack() as ctx:
            pools = {
                "in": ctx.enter_context(tc.tile_pool(name="inp", bufs=1)),
                "work": ctx.enter_context(tc.tile_pool(name="work", bufs=wbufs)),
                "scr": ctx.enter_context(tc.tile_pool(name="scr", bufs=3)),
                "stats": ctx.enter_context(tc.tile_pool(name="stats", bufs=1)),
                "ph2": ctx.enter_context(tc.tile_pool(name="ph2", bufs=1)),
            }
            pools["bH"] = bH
            pools["bC"] = bC
            pools["bF2"] = bF2
            in_sb = pools["in"].tile([128, tot], BF16, tag="in", name="in")
            cst = pools["stats"].tile([128, 2 * N_TILES], F32, tag="cst",
                                      name="cst")
            st = {
                "all": pools["stats"].tile([128, N_TILES, 15], F32,
                                           tag="st_all", name="st_all"),
                "ss": pools["stats"].tile([128, N_TILES], F32,
                                          tag="st_ss", name="st_ss"),
            }
            loss_tile = pools["ph2"].tile([128, N_TILES], F32, tag="loss",
                                          name="loss")

            # input DMAs: small first chunk so compute starts early, then
            # n_dma-1 even chunks over the rest; cst after the first chunk
            bounds = [0, 1]
            rem = N_MERGED - 1
            for d in range(n_dma - 1):
                bounds.append(1 + ((d + 1) * rem) // (n_dma - 1))
            first = True
            for ma, mb in zip(bounds[:-1], bounds[1:]):
                if ma >= mb:
                    continue
                e0 = offs[ma]
                e1 = offs[mb - 1] + 6 * GM * lmaxes[mb - 1]
                if first:
                    # stage the first chunk: one sub-tile, rest of tile, rest
                    eq = e0 + 6 * lmaxes[ma]
                    eh = e0 + 6 * GM * lmaxes[ma]
                    nc.sync.dma_start(out=in_sb[:, e0:eq], in_=pq[:, e0:eq])
                    nc.sync.dma_start(out=in_sb[:, eq:eh], in_=pq[:, eq:eh])
                    if eh < e1:
                        nc.sync.dma_start(out=in_sb[:, eh:e1],
                                          in_=pq[:, eh:e1])
                    nc.sync.dma_start(out=cst[:, :], in_=cstd)
                    first = False
                else:
                    nc.sync.dma_start(out=in_sb[:, e0:e1], in_=pq[:, e0:e1])

            ci = 0
            prods = {}
            prods[0] = _emit_products(tc, pools, in_sb, 0, lmaxes[0], offs[0])
            for m in range(N_MERGED):
                if m + 1 < N_MERGED:
                    prods[m + 1] = _emit_products(
                        tc, pools, in_sb, m + 1, lmaxes[m + 1], offs[m + 1])
                _phase1_rest(tc, pools, in_sb, st, m, lmaxes[m], offs[m],
                             prods.pop(m))
                while ci < len(chunks) and GM * (m + 1) >= chunks[ci][1]:
                    a, b = chunks[ci]

                    def _dma_out(x0, x1, a=a):
                        nc.sync.dma_start(out=loss[:, a + x0:a + x1],
                                          in_=loss_tile[:, a + x0:a + x1])
                    p2 = P2(tc, pools["ph2"], a, b, ci, dma_out=_dma_out)
                    _phase2(tc, p2, st, cst, loss_tile[:, a:b])
                    ci += 1
    _legalize_single_wait(nc)
    return nc


_nc_cache = {}


def _get_program(lmaxes, chunks=((0, 32),), n_dma=9, bH=28, bC=18, wbufs=3,
                 order="desc"):
    key = (lmaxes, chunks, n_dma, bH, bC, wbufs, order)
    if key not in _nc_cache:
        _nc_cache[key] = build_program(lmaxes, chunks, n_dma, bH, bC, wbufs,
                                       order)
    return _nc_cache[key]


def _prep(pred_coord, true_coord, pad_mask, torder="desc"):
    """Host-side packing. Returns (lmaxes, in_maps)."""
    P = np.asarray(pred_coord, dtype=np.float32)
    Q = np.asarray(true_coord, dtype=np.float32)
    M = np.asarray(pad_mask)
    B = P.shape[0]
    assert B == B_FULL and P.shape[1] == N_SEQ

    lengths = (N_SEQ - M.sum(axis=1)).astype(np.int64)
    order = np.argsort(lengths, kind="stable")
    lsort = lengths[order]
    # merged tile m takes sorted block blk[m]; longest first so the early
    # DMA chunks carry the most compute
    if torder == "ilv":
        blk = []
        hi, lo = N_MERGED - 1, N_MERGED // 2 - 1
        for i in range(N_MERGED // 2):
            blk.append(hi - i)
            blk.append(lo - i)
        blk = tuple(blk)
    else:
        blk = tuple(range(N_MERGED - 1, -1, -1))
    bsz = 1024 * GM
    lmaxes = []
    for m in range(N_MERGED):
        L = int(lsort[bsz * (blk[m] + 1) - 1])
        L = max(8, (L + 7) & ~7)
        lmaxes.append(L)
    lmaxes = tuple(lmaxes)

    # zero padding, transpose to [B, 3, N], sort
    w = (np.arange(N_SEQ)[None, :] < lengths[:, None]).astype(np.float32)
    Pz = (P * w[:, :, None]).transpose(0, 2, 1)[order]   # [B, 3, N]
    Qz = (Q * w[:, :, None]).transpose(0, 2, 1)[order]
    Pb = Pz.astype(ml_dtypes.bfloat16)
    Qb = Qz.astype(ml_dtypes.bfloat16)

    tot = sum(6 * GM * L for L in lmaxes)
    in_maps = []
    linv = (1.0 / lsort.astype(np.float64)).astype(np.float32)
    for c in range(N_CORES):
        buf = np.zeros((128, tot), dtype=ml_dtypes.bfloat16)
        o = 0
        for m in range(N_MERGED):
            L = lmaxes[m]
            bm = blk[m]
            gsel = np.arange(bsz * bm + c, bsz * (bm + 1), 8)  # 128*GM sorted
            Pm = Pb[gsel][:, :, :L]      # [128*GM, 3, L]
            Qm = Qb[gsel][:, :, :L]
            for g in range(GM):
                sl = slice(128 * g, 128 * (g + 1))
                buf[:, o:o + 3 * L] = Pm[sl].reshape(128, 3 * L)
                buf[:, o + 3 * L:o + 6 * L] = Qm[sl].reshape(128, 3 * L)
                o += 6 * L
        # constants: invn (32 cols), invn/3 (32 cols); col t, partition p
        # -> sorted index (t*128+p)*8 + c
        idx = (np.arange(B_CORE) * 8 + c)
        nin = linv[idx].reshape(N_TILES, 128).T          # [128, 32]
        # column t = sub-tile GM*m+g holds sorted sub-block GM*blk[m]+g
        perm = [GM * blk[t // GM] + (t % GM) for t in range(N_TILES)]
        nin = nin[:, perm]
        cstv = np.concatenate([nin, nin / 3.0], axis=1).astype(np.float32)
        in_maps.append({"pq": buf, "cst": np.ascontiguousarray(cstv)})
    return lmaxes, in_maps


def kernel(pred_coord, true_coord, pad_mask):
    lmaxes, in_maps = _prep(pred_coord, true_coord, pad_mask)
    nc = _get_program(lmaxes)
    trace = bool(int(os.environ.get("KERNEL_TRACE", "0")))
    res = run_bass_kernel_spmd(nc, in_maps, core_ids=list(range(N_CORES)),
                               trace=trace)
    if trace and res.exec_time_ns is not None:
        print(f"HW exec time: {res.exec_time_ns} ns")
        kernel.last_exec_time_ns = res.exec_time_ns
    total = 0.0
    for r in res.results:
        total += r["loss"].astype(np.float64).sum()
    return np.float32(total / B_FULL)


kernel.last_exec_time_ns = None


# revision 5
# speedup vs baseline: 1.0176x; 1.0025x over previous
"""Trainium2 Bass kernel v2: batched Kabsch-aligned masked MSE.

Math: per-sample loss = (|Pc|^2+|Qc|^2 - 2 t)/(3n) with t = s1+s2+sign(detH)*s3,
s_i = singular values of the 3x3 cross-covariance H = Pc^T Qc.  s_i^2 are the
eigenvalues of K = H^T H, found in closed form (Cardano / trigonometric method
using Arctan+Sin on the ACT engine).  No eigenvector needed.

Layout: samples sorted by valid length, striped over 8 cores; on-core 32 tiles
of 128 samples (samples on partitions), pairs of tiles merged (shared length
crop L).  Inputs are bf16, zero-padded on the host, shipped pre-transposed
as [P_A | Q_A | P_B | Q_B] per partition row, in a handful of large
contiguous DMAs.  Phase 1 computes per-sample sums (H, sp, sq, sppqq) with
DVE bf16 2x products + Pool folds + DVE reduces + ACT square-accum.  Phase 2
solves the 3x3 eigenproblem elementwise on [128, C] stat tiles.
"""

import os
import numpy as np
import ml_dtypes

import bass_rust
import concourse.bass as bass
import concourse.tile as tile
from concourse import mybir
from concourse.bass_utils import run_bass_kernel_spmd

F32 = mybir.dt.float32
BF16 = mybir.dt.bfloat16
Alu = mybir.AluOpType
Act = mybir.ActivationFunctionType
AX = mybir.AxisListType

N_CORES = 8
B_FULL = 32768
N_SEQ = 128
B_CORE = B_FULL // N_CORES      # 4096
N_TILES = B_CORE // 128         # 32 sub-tiles
GM = 4                          # sub-tiles per merged tile
N_MERGED = N_TILES // GM        # merged tiles
SQ3 = 1.7320508075688772
PI = 3.141592653589793


def _legalize_single_wait(nc):
    """Split multi-wait instructions into chains of single-wait Drains
    (deployed walrus build allows only one sync-wait per instruction)."""
    moved = 0
    for fn in nc.m.functions:
        for blk in fn.blocks:
            insts = blk.instructions
            new_list = []
            for ins in insts:
                si = ins.sync_info
                ow = list(si.on_wait) if si is not None and si.on_wait else []
                if len(ow) > 1:
                    for w in ow[:-1]:
                        d = mybir.InstDrain(name=f"I-sw{moved}", ins=[],
                                            outs=[], bass_is_fusable=False)
                        d.engine = ins.engine
                        d.sync_info = bass_rust.SyncInfo(on_wait=[w],
                                                         on_update=[])
                        new_list.append(d)
                        moved += 1
                    si.on_wait = [ow[-1]]
                new_list.append(ins)
            blk.instructions[:] = new_list
    return moved


def _ap(base, extra_offset, dims):
    """Manual AP: keep base's partition dim, replace free dims."""
    return bass.AP(tensor=base.tensor, offset=base.offset + extra_offset,
                   ap=[base.ap[0]] + [list(d) for d in dims])


def _emit_products(tc, pools, in_sb, m, L, off):
    """Products for merged tile m -> bf16 tile [p, 18, L], g-major blocks."""
    nc = tc.nc
    V = nc.vector
    prod = pools["work"].tile([128, 9 * GM * 128], BF16, tag="prod",
                              name="prod")
    for g in range(GM):
        p0 = off + 6 * L * g
        Pv = (in_sb[:, p0:p0 + 3 * L]
              .rearrange("p (i n) -> p i n", i=3)
              .unsqueeze(2).broadcast_to([128, 3, 3, L]))
        Qv = (in_sb[:, p0 + 3 * L:p0 + 6 * L]
              .rearrange("p (j n) -> p j n", j=3)
              .unsqueeze(1).broadcast_to([128, 3, 3, L]))
        out = prod[:, 9 * L * g:9 * L * (g + 1)].rearrange(
            "p (i j n) -> p i j n", i=3, j=3)
        V.tensor_tensor(out=out, in0=Pv, in1=Qv, op=Alu.mult)
    return prod


def _phase1_rest(tc, pools, in_sb, st, m, L, off, prod):
    """Folds + reduce + sppqq for merged tile m.

    Combined fold buffer blocks (30 x L2): [H_A(9) H_B(9) c_A(6) c_B(6)];
    three fold levels, then two TRs write st['all'][:, 2m:2m+2, :]
    (per sub-tile 15 = H(9), sp(3), sq(3)).
    """
    nc = tc.nc
    V, G, A = nc.vector, nc.gpsimd, nc.scalar
    L2, L4, L8 = L // 2, L // 4, L // 8
    bH = pools.get("bH", 28)    # of 9*GM H-fold blocks on Pool
    bC = pools.get("bC", 18)    # of 6*GM c-fold blocks on Pool
    if L <= pools.get("poolmin", 0):
        bH = bC = 0             # short tiles: avoid cross-engine latency

    NB = 15 * GM
    NH = 9 * GM
    NC = 6 * GM
    fb = pools["work"].tile([128, NB * 64], BF16, tag="fold", name="fold")
    fb2 = pools["work"].tile([128, NB * 32], BF16, tag="fold2", name="fold2")
    fb3 = pools["work"].tile([128, NB * 16], BF16, tag="fold3", name="fold3")
    ascr = pools["scr"].tile([128, 6 * 128], BF16, tag="ascr", name="ascr")

    fv = fb[:, 0:NB * L2].rearrange("p (k n) -> p k n", k=NB)
    fv2 = fb2[:, 0:NB * L4].rearrange("p (k n) -> p k n", k=NB)
    fv3 = fb3[:, 0:NB * L8].rearrange("p (k n) -> p k n", k=NB)
    pv = prod[:, 0:NH * L].rearrange("p (k n) -> p k n", k=NH)
    iv = in_sb[:, off:off + NC * L].rearrange("p (k n) -> p k n", k=NC)

    # fold1: H blocks [0,NH) from prod, c blocks [NH,NB) from input;
    # first bH/bC blocks on Pool, rest on DVE
    for dst0, srcv, nblk, npool in ((0, pv, NH, bH), (NH, iv, NC, bC)):
        for eng, k0, k1 in ((G, 0, npool), (V, npool, nblk)):
            if k0 >= k1:
                continue
            eng.tensor_tensor(
                out=fv[:, dst0 + k0:dst0 + k1, :],
                in0=srcv[:, k0:k1, 0:L2],
                in1=srcv[:, k0:k1, L2:2 * L2],
                op=Alu.add)

    # extra fold levels while profitable (halving pays iff width/2 >= 4)
    bF2 = pools.get("bF2", 0)   # fold2 blocks on Pool
    last = fv
    width = L2
    for lvl, nxt in enumerate((fv2, fv3)):
        if width // 2 < 4:
            break
        w2 = width // 2
        npool = bF2 if lvl == 0 else 0
        for eng, k0, k1 in ((G, 0, npool), (V, npool, NB)):
            if k0 >= k1:
                continue
            eng.tensor_tensor(out=nxt[:, k0:k1, 0:w2],
                              in0=last[:, k0:k1, 0:w2],
                              in1=last[:, k0:k1, w2:width], op=Alu.add)
        last, width = nxt, w2
    st3 = st["all"][:, :, :]
    outH = _ap(st3, 15 * GM * m, [[15, GM], [1, 9]])
    V.tensor_reduce(out=outH, in_=last[:, 0:NH, 0:width], axis=AX.X,
                    op=Alu.add)
    outC = _ap(st3, 15 * GM * m + 9, [[15, GM], [1, 6]])
    V.tensor_reduce(out=outC, in_=last[:, NH:NB, 0:width], axis=AX.X,
                    op=Alu.add)

    # sppqq per sub-tile: ACT square with accumulate over [p, 6L]
    for g in range(GM):
        p0 = off + 6 * L * g
        t = GM * m + g
        A.activation(out=ascr[:, 0:6 * L], in_=in_sb[:, p0:p0 + 6 * L],
                     func=Act.Square,
                     accum_out=st["ss"][:, t:t + 1])


class P2:
    """Emit elementwise phase-2 ops on [128, C] column tiles."""

    def __init__(self, tc, pool, c0, c1, chunk, dma_out=None):
        self.nc = tc.nc
        self.pool = pool
        self.c0, self.c1 = c0, c1
        self.C = c1 - c0
        self.chunk = chunk
        self.ctr = 0
        self.dma_out = dma_out

    def mk(self, name=None):
        self.ctr += 1
        tag = f"c{self.chunk}_" + (name or f"t{self.ctr}")
        return self.pool.tile([128, self.C], F32, tag=tag, name=tag)

    def tt(self, a, b, op, eng=None, out=None):
        dst = out if out is not None else self.mk()
        (eng or self.nc.vector).tensor_tensor(out=dst, in0=a, in1=b, op=op)
        return dst

    def mul(self, a, b, eng=None, out=None):
        return self.tt(a, b, Alu.mult, eng, out)

    def add(self, a, b, eng=None, out=None):
        return self.tt(a, b, Alu.add, eng, out)

    def sub(self, a, b, eng=None, out=None):
        return self.tt(a, b, Alu.subtract, eng, out)

    def ts(self, a, s1, op0, s2=None, op1=Alu.bypass, eng=None, out=None):
        dst = out if out is not None else self.mk()
        (eng or self.nc.vector).tensor_scalar(
            out=dst, in0=a, scalar1=s1, scalar2=s2, op0=op0, op1=op1)
        return dst

    def stt(self, a, s, b, op0, op1, eng=None, out=None):
        """(a op0 s) op1 b in one instruction."""
        dst = out if out is not None else self.mk()
        (eng or self.nc.vector).scalar_tensor_tensor(
            out=dst, in0=a, scalar=s, in1=b, op0=op0, op1=op1)
        return dst

    def recip(self, a, out=None):
        dst = out if out is not None else self.mk()
        self.nc.vector.reciprocal(out=dst, in_=a)
        return dst

    def act(self, a, func, bias=0.0, scale=1.0, out=None):
        dst = out if out is not None else self.mk()
        self.nc.scalar.activation(out=dst, in_=a, func=func, bias=bias,
                                  scale=scale)
        return dst


def _phase2(tc, p2, st, cst, loss_out):
    """Per-sample Kabsch loss from stats, columns [c0, c1) (c = sub-tile).

    t = lam + 4*wx(lam)/p'(lam); lam = s1+s2+d*s3 via Cardano on K = Hc^T Hc;
    p'(lam) = 8(s2+d*s3)(s1+d*s3)(s1+s2); wx(lam) = -|a|^2 lam^2 + Wb lam + Wc
    is the adjugate-row-0 dot product, coefficients lam-free (computed early,
    off the critical path).
    """
    nc = tc.nc
    V, G, A = nc.vector, nc.gpsimd, nc.scalar
    c0, C = p2.c0, p2.C

    St = st["all"][:, :, :]        # [p, 32, 15]
    H9 = _ap(St, 15 * c0, [[15, C], [3, 3], [1, 3]])     # [p, c, i, j]
    sp_b = _ap(St, 15 * c0 + 9, [[15, C], [1, 3], [0, 3]])
    ss = st["ss"][:, c0:c0 + C]
    invn = cst[:, c0:c0 + C]
    invn3 = cst[:, N_TILES + c0:N_TILES + c0 + C]

    def wide(name, k):
        tag = f"c{p2.chunk}_{name}"
        return p2.pool.tile([128, C * k], F32, tag=tag, name=tag)

    # spqn = spq * invn (6-wide); corr = sum(spq*spqn); ppqqc = ss - corr
    spq6 = _ap(St, 15 * c0 + 9, [[15, C], [1, 6]])
    spqn6 = wide("spqn6", 6)
    spqn6_v = spqn6[:, :].rearrange("p (c k) -> p c k", k=6)
    inb6 = invn[:, :].unsqueeze(2).broadcast_to([128, C, 6])
    V.tensor_tensor(out=spqn6_v, in0=spq6, in1=inb6, op=Alu.mult)
    corrp = wide("corrp", 6)
    corrp_v = corrp[:, :].rearrange("p (c k) -> p c k", k=6)
    G.tensor_tensor(out=corrp_v, in0=spq6, in1=spqn6_v, op=Alu.mult)
    corr = p2.mk("corr")
    V.tensor_reduce(out=corr, in_=corrp_v, axis=AX.X, op=Alu.add)
    ppqqc = p2.sub(ss, corr, G)

    # centering: Hc[c, i, j] = H - sp_i * sqn_j
    mv = wide("mv", 9)
    mv_v = mv[:, :].rearrange("p (c i j) -> p c i j", i=3, j=3)
    sqn_b = bass.AP(tensor=spqn6_v.tensor, offset=spqn6_v.offset + 3,
                    ap=[spqn6_v.ap[0], [6, C], [0, 3], [1, 3]])
    V.tensor_tensor(out=mv_v, in0=sp_b, in1=sqn_b, op=Alu.mult)
    Hc = wide("Hc", 9)
    Hc_v = Hc[:, :].rearrange("p (c k) -> p c k", k=9)
    H9f = _ap(St, 15 * c0, [[15, C], [1, 9]])
    V.tensor_tensor(out=Hc_v, in0=H9f, in1=mv[:, :].rearrange(
        "p (c k) -> p c k", k=9), op=Alu.subtract)
    hc0 = Hc[:, :]
    h = {(i, j): _ap(hc0, 3 * i + j, [[9, C]]) for i in range(3)
         for j in range(3)}

    # K = Hc^T Hc: 3 products into one (c,a,b,i) tile, single reduce
    Kt = wide("Kt", 9)
    kp = wide("kp", 27)
    for aa in range(3):
        in0 = _ap(hc0, aa, [[9, C], [0, 3], [3, 3]])
        in1 = _ap(hc0, 0, [[9, C], [1, 3], [3, 3]])
        kp_v = _ap(kp[:, :], 9 * aa, [[27, C], [3, 3], [1, 3]])
        V.tensor_tensor(out=kp_v, in0=in0, in1=in1, op=Alu.mult)
    kp_flat = _ap(kp[:, :], 0, [[3, 9 * C], [1, 3]])
    V.tensor_reduce(out=Kt[:, :], in_=kp_flat, axis=AX.X, op=Alu.add)
    trK = p2.mk("trK")
    diag_v = _ap(Kt[:, :], 0, [[9, C], [4, 3]])
    V.tensor_reduce(out=trK, in_=diag_v, axis=AX.X, op=Alu.add)
    k2 = wide("k2", 9)
    V.tensor_tensor(out=k2[:, :], in0=Kt[:, :], in1=Kt[:, :], op=Alu.mult)
    trK2 = p2.mk("trK2")
    V.tensor_reduce(out=trK2, in_=k2[:, :].rearrange("p (c k) -> p c k", k=9),
                    axis=AX.X, op=Alu.add)

    # detH (of Hc) via 2x2 minors (Pool, off-spine)
    def minor2(pq, qq, rq_, sq_, eng=G):
        t1 = p2.mul(pq, qq, eng)
        t2 = p2.mul(rq_, sq_, eng)
        return p2.sub(t1, t2, eng)

    mm1 = minor2(h[(1, 1)], h[(2, 2)], h[(1, 2)], h[(2, 1)])
    mm2 = minor2(h[(1, 0)], h[(2, 2)], h[(1, 2)], h[(2, 0)])
    mm3 = minor2(h[(1, 0)], h[(2, 1)], h[(1, 1)], h[(2, 0)])
    dd1 = p2.mul(h[(0, 0)], mm1, G)
    dd2 = p2.mul(h[(0, 1)], mm2, G)
    dd3 = p2.mul(h[(0, 2)], mm3, G)
    detH = p2.add(p2.sub(dd1, dd2, G), dd3, G)
    sgn = p2.act(detH, Act.Sign)
    detK = p2.act(detH, Act.Square)

    # --- Cardano spine starts (DVE), W-coefficient work interleaved into
    # the spine's dependency-stall windows ---
    trKsq = p2.mul(trK, trK, V)
    p6 = p2.stt(trKsq, -1.0 / 3.0, trK2, Alu.mult, Alu.add, V)
    p6c = p2.ts(p6, 1e-12, Alu.max, eng=V)
    sqp = p2.act(p6c, Act.Sqrt, scale=1.0 / 6.0)         # sqrt(p)
    mmean = p2.ts(trK, 1.0 / 3.0, Alu.mult, eng=V)

    # [fill] Horn matrix entries of M = Hc^T, packed for one-shot squares
    npk1 = p2.pool.tile([128, 3 * C], F32, tag=f"c{p2.chunk}_npk1",
                        name="npk1")
    npk2 = p2.pool.tile([128, 3 * C], F32, tag=f"c{p2.chunk}_npk2",
                        name="npk2")
    n01 = p2.sub(h[(2, 1)], h[(1, 2)], G, out=npk1[:, 0:C])
    n02 = p2.sub(h[(0, 2)], h[(2, 0)], G, out=npk1[:, C:2 * C])
    n03 = p2.sub(h[(1, 0)], h[(0, 1)], G, out=npk1[:, 2 * C:3 * C])
    n23 = p2.add(h[(2, 1)], h[(1, 2)], V, out=npk2[:, 0:C])
    n13 = p2.add(h[(0, 2)], h[(2, 0)], V, out=npk2[:, C:2 * C])
    n12 = p2.add(h[(1, 0)], h[(0, 1)], V, out=npk2[:, 2 * C:3 * C])

    msq = p2.ts(trKsq, 1.0 / 9.0, Alu.mult, eng=V)
    m3c = p2.mul(msq, mmean, G)
    u = p2.stt(detK, 0.5, m3c, Alu.mult, Alu.add, V)     # m^3 + detK/2
    tdiff = p2.sub(trKsq, trK2, V)                       # 2*M2
    tm = p2.mul(tdiff, mmean, V)
    q = p2.stt(tm, -0.25, u, Alu.mult, Alu.add, V)
    p6sq = p2.mul(p6c, p6c, V)
    p3 = p2.mul(p6sq, p6c, V)
    q2 = p2.mul(q, q, V)
    pfloor = p2.ts(p3, 9.26e-11, Alu.mult, 1e-38, Alu.max, eng=V)
    diff = p2.stt(p3, 1.0 / 216.0, q2, Alu.mult, Alu.subtract, V)
    diffc = p2.tt(diff, pfloor, Alu.max, V)
    sqd = p2.act(diffc, Act.Sqrt)

    # [fill] squares of the packed entries + first W terms
    usq = p2.pool.tile([128, 3 * C], F32, tag=f"c{p2.chunk}_usq", name="usq")
    A.activation(out=usq[:, :], in_=npk1[:, :], func=Act.Square)
    u1, u2, u3 = usq[:, 0:C], usq[:, C:2 * C], usq[:, 2 * C:3 * C]
    wsq = p2.pool.tile([128, 3 * C], F32, tag=f"c{p2.chunk}_wsq", name="wsq")
    A.activation(out=wsq[:, :], in_=npk2[:, :], func=Act.Square)
    n23s, n13s, n12s = wsq[:, 0:C], wsq[:, C:2 * C], wsq[:, 2 * C:3 * C]
    tr3 = p2.add(p2.add(h[(0, 0)], h[(1, 1)], V), h[(2, 2)], V)
    n11 = p2.stt(h[(0, 0)], 2.0, tr3, Alu.mult, Alu.subtract, V)
    n22 = p2.stt(h[(1, 1)], 2.0, tr3, Alu.mult, Alu.subtract, V)
    n33 = p2.stt(h[(2, 2)], 2.0, tr3, Alu.mult, Alu.subtract, V)
    v1 = p2.mul(n01, n02, G)
    v2 = p2.mul(n01, n03, G)
    v3 = p2.mul(n02, n03, G)

    rq = p2.recip(sqd)
    ratio = p2.mul(q, rq, V)
    ratioc = p2.ts(ratio, 100.0, Alu.min, -100.0, Alu.max, V)
    at = p2.act(ratioc, Act.Arctan)

    # [fill] Wa, Wb
    Wa_n = p2.add(p2.add(u1, u2, V), u3, V)
    s1s = p2.add(n22, n33, G)
    s2s = p2.add(n11, n33, G)
    s3s = p2.add(n11, n22, G)
    b1 = p2.mul(u1, s1s, V)
    b2 = p2.mul(u2, s2s, V)
    b3 = p2.mul(u3, s3s, V)
    b4 = p2.mul(v1, n12, G)
    b5 = p2.mul(v2, n13, G)
    b6 = p2.mul(v3, n23, G)

    # cos/sin of phi straight from `at` on ACT (func(scale*x+bias)):
    # stays on the ACT queue, no DVE round-trip
    cphi = p2.act(at, Act.Sin, bias=PI / 6.0 + PI / 2.0, scale=-1.0 / 3.0)
    sphi = p2.act(at, Act.Sin, bias=PI / 6.0, scale=-1.0 / 3.0)

    # [fill] Wb finish, Wc terms
    a123 = p2.add(p2.add(b1, b2, V), b3, V)
    c456 = p2.add(p2.add(b4, b5, G), b6, G)
    Wb = p2.stt(c456, -2.0, a123, Alu.mult, Alu.add, V)
    M1 = p2.sub(p2.mul(n22, n33, G), n23s, G)
    M2m = p2.sub(p2.mul(n11, n33, G), n13s, G)
    M3m = p2.sub(p2.mul(n11, n22, G), n12s, G)

    # eigenvalues via mp +/- sqrt(3)*ps; one packed Sqrt for all three
    pc = p2.mul(sqp, cphi, V)
    ps = p2.mul(sqp, sphi, V)
    lamp = p2.pool.tile([128, 3 * C], F32, tag=f"c{p2.chunk}_lamp",
                        name="lamp")
    p2.stt(pc, 2.0, mmean, Alu.mult, Alu.add, V, out=lamp[:, 0:C])
    mp = p2.sub(mmean, pc, V)
    s3p = p2.ts(ps, SQ3, Alu.mult, eng=V)
    lam2 = p2.add(mp, s3p, V)
    p2.ts(lam2, 0.0, Alu.max, eng=V, out=lamp[:, C:2 * C])
    lam3 = p2.sub(mp, s3p, V)
    p2.ts(lam3, 0.0, Alu.max, eng=V, out=lamp[:, 2 * C:3 * C])
    sgt = p2.pool.tile([128, 3 * C], F32, tag=f"c{p2.chunk}_sgt", name="sgt")
    A.activation(out=sgt[:, :], in_=lamp[:, :], func=Act.Sqrt)
    sg1 = sgt[:, 0:C]
    sg2 = sgt[:, C:2 * C]
    sg3 = sgt[:, 2 * C:3 * C]

    # [fill] Wc finish
    dd_ = p2.add(p2.add(p2.mul(u1, M1, G), p2.mul(u2, M2m, G), G),
                 p2.mul(u3, M3m, G), G)
    cc1 = p2.sub(p2.mul(n12, n33, V), p2.mul(n13, n23, V), V)
    cc2 = p2.sub(p2.mul(n12, n23, V), p2.mul(n13, n22, V), V)
    cc3 = p2.sub(p2.mul(n11, n23, V), p2.mul(n12, n13, V), V)
    ee = p2.add(p2.sub(p2.mul(v1, cc1, V), p2.mul(v2, cc2, V), V),
                p2.mul(v3, cc3, V), V)
    Wc = p2.stt(ee, 2.0, dd_, Alu.mult, Alu.subtract, V)   # 2*ee - dd

    s3d = p2.mul(sgn, sg3, V)
    t12 = p2.add(sg1, sg2, V)
    lam = p2.add(t12, s3d, V)                            # lambda_max of Horn
    pp1 = p2.add(sg2, s3d, V)
    pp2 = p2.add(sg1, s3d, V)
    ppr = p2.mul(pp1, pp2, V)
    ppr2 = p2.mul(ppr, t12, V)                           # p'(lam)/8
    pprc = p2.ts(ppr2, 1e-13, Alu.max, eng=V)
    rp = p2.recip(pprc)

    # wx = (-Wa_n*lam + Wb)*lam + Wc, then t and the loss
    wt1 = p2.mul(Wa_n, lam, V)
    wt2 = p2.sub(Wb, wt1, V)
    wt3 = p2.mul(wt2, lam, V)
    wx_v = p2.add(wt3, Wc, V)
    corr4 = p2.mul(wx_v, rp, V)
    t_unc = p2.stt(corr4, 0.5, lam, Alu.mult, Alu.add, V)  # lam + 4wx/p'
    ssum = p2.add(t12, sg3, V)
    tb = p2.tt(t_unc, ssum, Alu.min, V)
    ssn = p2.ts(ssum, -1.0, Alu.mult, eng=V)
    tcl = p2.tt(tb, ssn, Alu.max, V)
    li = p2.stt(tcl, -2.0, ppqqc, Alu.mult, Alu.add, V)
    p2.mul(li, invn3, V, out=loss_out)
    if p2.dma_out is not None:
        p2.dma_out(0, C)


def build_program(lmaxes, chunks=((0, 32),), n_dma=9, bH=28, bC=18,
                  wbufs=3, order="desc", bF2=0):
    """lmaxes: per-merged-tile crop lengths (16 ints, multiples of 4)."""
    assert len(lmaxes) == N_MERGED
    tot = sum(6 * GM * L for L in lmaxes)
    offs = []
    o = 0
    for L in lmaxes:
        offs.append(o)
        o += 6 * GM * L

    nc = bass.Bass("TRN2", debug=False, enable_asserts=False,
                   target_bir_lowering=False)
    # extra activation-bias constants (only 0.0/1.0 pre-registered)
    for cval in (PI / 2.0, PI / 6.0 + PI / 2.0, PI / 6.0):
        cten = nc.alloc_sbuf_tensor(f"const-f32-{cval}", [128, 1], F32)
        nc.gpsimd.memset(cten.ap(), cval)
        nc.const_aps.aps[(F32, cval)] = cten.ap()
    nc.all_engine_barrier()
    pq = nc.dram_tensor("pq", [128, tot], BF16, kind="ExternalInput").ap()
    cstd = nc.dram_tensor("cst", [128, 2 * N_TILES], F32,
                          kind="ExternalInput").ap()
    loss = nc.dram_tensor("loss", [128, N_TILES], F32,
                          kind="ExternalOutput").ap()

    with tile.TileContext(nc) as tc:
        from contextlib import ExitStack
        with ExitStack() as ctx:
            pools = {
                "in": ctx.enter_context(tc.tile_pool(name="inp", bufs=1)),
                "work": ctx.enter_context(tc.tile_pool(name="work", bufs=wbufs)),
                "scr": ctx.enter_context(tc.tile_pool(name="scr", bufs=3)),
                "stats": ctx.enter_context(tc.tile_pool(name="stats", bufs=1)),
                "ph2": ctx.enter_context(tc.tile_pool(name="ph2", bufs=1)),
            }
            pools["bH"] = bH
            pools["bC"] = bC
            pools["bF2"] = bF2
            in_sb = pools["in"].tile([128, tot], BF16, tag="in", name="in")
            cst = pools["stats"].tile([128, 2 * N_TILES], F32, tag="cst",
                                      name="cst")
            st = {
                "all": pools["stats"].tile([128, N_TILES, 15], F32,
                                           tag="st_all", name="st_all"),
                "ss": pools["stats"].tile([128, N_TILES], F32,
                                          tag="st_ss", name="st_ss"),
            }
            loss_tile = pools["ph2"].tile([128, N_TILES], F32, tag="loss",
                                          name="loss")

            # input DMAs: small first chunk so compute starts early, then
            # n_dma-1 even chunks over the rest; cst after the first chunk
            bounds = [0, 1]
            rem = N_MERGED - 1
            for d in range(n_dma - 1):
                bounds.append(1 + ((d + 1) * rem) // (n_dma - 1))
            first = True
            for ma, mb in zip(bounds[:-1], bounds[1:]):
                if ma >= mb:
                    continue
                e0 = offs[ma]
                e1 = offs[mb - 1] + 6 * GM * lmaxes[mb - 1]
                if first:
                    # stage the first chunk: one sub-tile, rest of tile, rest
                    eq = e0 + 6 * lmaxes[ma]
                    eh = e0 + 6 * GM * lmaxes[ma]
                    nc.sync.dma_start(out=in_sb[:, e0:eq], in_=pq[:, e0:eq])
                    nc.sync.dma_start(out=in_sb[:, eq:eh], in_=pq[:, eq:eh])
                    if eh < e1:
                        nc.sync.dma_start(out=in_sb[:, eh:e1],
                                          in_=pq[:, eh:e1])
                    first = False
                else:
                    nc.sync.dma_start(out=in_sb[:, e0:e1], in_=pq[:, e0:e1])
            # constants are first needed by phase 2 -> keep them out of the
            # ramp-critical part of the input stream
            nc.sync.dma_start(out=cst[:, :], in_=cstd)

            ci = 0
            prods = {}
            prods[0] = _emit_products(tc, pools, in_sb, 0, lmaxes[0], offs[0])
            for m in range(N_MERGED):
                if m + 1 < N_MERGED:
                    prods[m + 1] = _emit_products(
                        tc, pools, in_sb, m + 1, lmaxes[m + 1], offs[m + 1])
                _phase1_rest(tc, pools, in_sb, st, m, lmaxes[m], offs[m],
                             prods.pop(m))
                while ci < len(chunks) and GM * (m + 1) >= chunks[ci][1]:
                    a, b = chunks[ci]

                    def _dma_out(x0, x1, a=a):
                        nc.sync.dma_start(out=loss[:, a + x0:a + x1],
                                          in_=loss_tile[:, a + x0:a + x1])
                    p2 = P2(tc, pools["ph2"], a, b, ci, dma_out=_dma_out)
                    _phase2(tc, p2, st, cst, loss_tile[:, a:b])
                    ci += 1
    _legalize_single_wait(nc)
    return nc


_nc_cache = {}


def _get_program(lmaxes, chunks=((0, 32),), n_dma=9, bH=28, bC=18, wbufs=3,
                 order="desc"):
    key = (lmaxes, chunks, n_dma, bH, bC, wbufs, order)
    if key not in _nc_cache:
        _nc_cache[key] = build_program(lmaxes, chunks, n_dma, bH, bC, wbufs,
                                       order)
    return _nc_cache[key]


def _prep(pred_coord, true_coord, pad_mask, torder="desc"):
    """Host-side packing. Returns (lmaxes, in_maps)."""
    P = np.asarray(pred_coord, dtype=np.float32)
    Q = np.asarray(true_coord, dtype=np.float32)
    M = np.asarray(pad_mask)
    B = P.shape[0]
    assert B == B_FULL and P.shape[1] == N_SEQ

    lengths = (N_SEQ - M.sum(axis=1)).astype(np.int64)
    order = np.argsort(lengths, kind="stable")
    lsort = lengths[order]
    # merged tile m takes sorted block blk[m]; longest first so the early
    # DMA chunks carry the most compute
    if torder == "ilv":
        blk = []
        hi, lo = N_MERGED - 1, N_MERGED // 2 - 1
        for i in range(N_MERGED // 2):
            blk.append(hi - i)
            blk.append(lo - i)
        blk = tuple(blk)
    else:
        blk = tuple(range(N_MERGED - 1, -1, -1))
    bsz = 1024 * GM
    lmaxes = []
    for m in range(N_MERGED):
        L = int(lsort[bsz * (blk[m] + 1) - 1])
        L = max(8, (L + 7) & ~7)
        lmaxes.append(L)
    lmaxes = tuple(lmaxes)

    # zero padding, transpose to [B, 3, N], sort
    w = (np.arange(N_SEQ)[None, :] < lengths[:, None]).astype(np.float32)
    Pz = (P * w[:, :, None]).transpose(0, 2, 1)[order]   # [B, 3, N]
    Qz = (Q * w[:, :, None]).transpose(0, 2, 1)[order]
    Pb = Pz.astype(ml_dtypes.bfloat16)
    Qb = Qz.astype(ml_dtypes.bfloat16)

    tot = sum(6 * GM * L for L in lmaxes)
    in_maps = []
    linv = (1.0 / lsort.astype(np.float64)).astype(np.float32)
    for c in range(N_CORES):
        buf = np.zeros((128, tot), dtype=ml_dtypes.bfloat16)
        o = 0
        for m in range(N_MERGED):
            L = lmaxes[m]
            bm = blk[m]
            gsel = np.arange(bsz * bm + c, bsz * (bm + 1), 8)  # 128*GM sorted
            Pm = Pb[gsel][:, :, :L]      # [128*GM, 3, L]
            Qm = Qb[gsel][:, :, :L]
            for g in range(GM):
                sl = slice(128 * g, 128 * (g + 1))
                buf[:, o:o + 3 * L] = Pm[sl].reshape(128, 3 * L)
                buf[:, o + 3 * L:o + 6 * L] = Qm[sl].reshape(128, 3 * L)
                o += 6 * L
        # constants: invn (32 cols), invn/3 (32 cols); col t, partition p
        # -> sorted index (t*128+p)*8 + c
        idx = (np.arange(B_CORE) * 8 + c)
        nin = linv[idx].reshape(N_TILES, 128).T          # [128, 32]
        # column t = sub-tile GM*m+g holds sorted sub-block GM*blk[m]+g
        perm = [GM * blk[t // GM] + (t % GM) for t in range(N_TILES)]
        nin = nin[:, perm]
        cstv = np.concatenate([nin, nin / 3.0], axis=1).astype(np.float32)
        in_maps.append({"pq": buf, "cst": np.ascontiguousarray(cstv)})
    return lmaxes, in_maps


def kernel(pred_coord, true_coord, pad_mask):
    lmaxes, in_maps = _prep(pred_coord, true_coord, pad_mask)
    nc = _get_program(lmaxes)
    trace = bool(int(os.environ.get("KERNEL_TRACE", "0")))
    res = run_bass_kernel_spmd(nc, in_maps, core_ids=list(range(N_CORES)),
                               trace=trace)
    if trace and res.exec_time_ns is not None:
        print(f"HW exec time: {res.exec_time_ns} ns")
        kernel.last_exec_time_ns = res.exec_time_ns
    total = 0.0
    for r in res.results:
        total += r["loss"].astype(np.float64).sum()
    return np.float32(total / B_FULL)


kernel.last_exec_time_ns = None


# revision 6
# speedup vs baseline: 1.0191x; 1.0014x over previous
"""Trainium2 Bass kernel v2: batched Kabsch-aligned masked MSE.

Math: per-sample loss = (|Pc|^2+|Qc|^2 - 2 t)/(3n) with t = s1+s2+sign(detH)*s3,
s_i = singular values of the 3x3 cross-covariance H = Pc^T Qc.  s_i^2 are the
eigenvalues of K = H^T H, found in closed form (Cardano / trigonometric method
using Arctan+Sin on the ACT engine).  No eigenvector needed.

Layout: samples sorted by valid length, striped over 8 cores; on-core 32 tiles
of 128 samples (samples on partitions), pairs of tiles merged (shared length
crop L).  Inputs are bf16, zero-padded on the host, shipped pre-transposed
as [P_A | Q_A | P_B | Q_B] per partition row, in a handful of large
contiguous DMAs.  Phase 1 computes per-sample sums (H, sp, sq, sppqq) with
DVE bf16 2x products + Pool folds + DVE reduces + ACT square-accum.  Phase 2
solves the 3x3 eigenproblem elementwise on [128, C] stat tiles.
"""

import os
import numpy as np
import ml_dtypes

import bass_rust
import concourse.bass as bass
import concourse.tile as tile
from concourse import mybir
from concourse.bass_utils import run_bass_kernel_spmd

F32 = mybir.dt.float32
BF16 = mybir.dt.bfloat16
Alu = mybir.AluOpType
Act = mybir.ActivationFunctionType
AX = mybir.AxisListType

N_CORES = 8
B_FULL = 32768
N_SEQ = 128
B_CORE = B_FULL // N_CORES      # 4096
N_TILES = B_CORE // 128         # 32 sub-tiles
GM = 4                          # sub-tiles per merged tile
N_MERGED = N_TILES // GM        # merged tiles
SQ3 = 1.7320508075688772
PI = 3.141592653589793


def _legalize_single_wait(nc):
    """Split multi-wait instructions into chains of single-wait Drains
    (deployed walrus build allows only one sync-wait per instruction)."""
    moved = 0
    for fn in nc.m.functions:
        for blk in fn.blocks:
            insts = blk.instructions
            new_list = []
            for ins in insts:
                si = ins.sync_info
                ow = list(si.on_wait) if si is not None and si.on_wait else []
                if len(ow) > 1:
                    for w in ow[:-1]:
                        d = mybir.InstDrain(name=f"I-sw{moved}", ins=[],
                                            outs=[], bass_is_fusable=False)
                        d.engine = ins.engine
                        d.sync_info = bass_rust.SyncInfo(on_wait=[w],
                                                         on_update=[])
                        new_list.append(d)
                        moved += 1
                    si.on_wait = [ow[-1]]
                new_list.append(ins)
            blk.instructions[:] = new_list
    return moved


def _ap(base, extra_offset, dims):
    """Manual AP: keep base's partition dim, replace free dims."""
    return bass.AP(tensor=base.tensor, offset=base.offset + extra_offset,
                   ap=[base.ap[0]] + [list(d) for d in dims])


def _emit_products(tc, pools, in_sb, m, L, off):
    """Products for merged tile m -> bf16 tile [p, 18, L], g-major blocks."""
    nc = tc.nc
    V = nc.vector
    prod = pools["work"].tile([128, 9 * GM * 128], BF16, tag="prod",
                              name="prod")
    for g in range(GM):
        p0 = off + 6 * L * g
        Pv = (in_sb[:, p0:p0 + 3 * L]
              .rearrange("p (i n) -> p i n", i=3)
              .unsqueeze(2).broadcast_to([128, 3, 3, L]))
        Qv = (in_sb[:, p0 + 3 * L:p0 + 6 * L]
              .rearrange("p (j n) -> p j n", j=3)
              .unsqueeze(1).broadcast_to([128, 3, 3, L]))
        out = prod[:, 9 * L * g:9 * L * (g + 1)].rearrange(
            "p (i j n) -> p i j n", i=3, j=3)
        V.tensor_tensor(out=out, in0=Pv, in1=Qv, op=Alu.mult)
    return prod


def _phase1_rest(tc, pools, in_sb, st, m, L, off, prod):
    """Folds + reduce + sppqq for merged tile m.

    Combined fold buffer blocks (30 x L2): [H_A(9) H_B(9) c_A(6) c_B(6)];
    three fold levels, then two TRs write st['all'][:, 2m:2m+2, :]
    (per sub-tile 15 = H(9), sp(3), sq(3)).
    """
    nc = tc.nc
    V, G, A = nc.vector, nc.gpsimd, nc.scalar
    L2, L4, L8 = L // 2, L // 4, L // 8
    bH = pools.get("bH", 28)    # of 9*GM H-fold blocks on Pool
    bC = pools.get("bC", 18)    # of 6*GM c-fold blocks on Pool
    if L <= pools.get("poolmin", 0):
        bH = bC = 0             # short tiles: avoid cross-engine latency

    NB = 15 * GM
    NH = 9 * GM
    NC = 6 * GM
    fb = pools["work"].tile([128, NB * 64], BF16, tag="fold", name="fold")
    fb2 = pools["work"].tile([128, NB * 32], BF16, tag="fold2", name="fold2")
    fb3 = pools["work"].tile([128, NB * 16], BF16, tag="fold3", name="fold3")
    ascr = pools["scr"].tile([128, 6 * 128], BF16, tag="ascr", name="ascr")

    fv = fb[:, 0:NB * L2].rearrange("p (k n) -> p k n", k=NB)
    fv2 = fb2[:, 0:NB * L4].rearrange("p (k n) -> p k n", k=NB)
    fv3 = fb3[:, 0:NB * L8].rearrange("p (k n) -> p k n", k=NB)
    pv = prod[:, 0:NH * L].rearrange("p (k n) -> p k n", k=NH)
    iv = in_sb[:, off:off + NC * L].rearrange("p (k n) -> p k n", k=NC)

    # fold1: H blocks [0,NH) from prod, c blocks [NH,NB) from input;
    # first bH/bC blocks on Pool, rest on DVE
    for dst0, srcv, nblk, npool in ((0, pv, NH, bH), (NH, iv, NC, bC)):
        for eng, k0, k1 in ((G, 0, npool), (V, npool, nblk)):
            if k0 >= k1:
                continue
            eng.tensor_tensor(
                out=fv[:, dst0 + k0:dst0 + k1, :],
                in0=srcv[:, k0:k1, 0:L2],
                in1=srcv[:, k0:k1, L2:2 * L2],
                op=Alu.add)

    # extra fold levels while profitable (halving pays iff width/2 >= 4)
    bF2 = pools.get("bF2", 0)   # fold2 blocks on Pool
    last = fv
    width = L2
    for lvl, nxt in enumerate((fv2, fv3)):
        if width // 2 < 4:
            break
        w2 = width // 2
        npool = bF2 if lvl == 0 else 0
        for eng, k0, k1 in ((G, 0, npool), (V, npool, NB)):
            if k0 >= k1:
                continue
            eng.tensor_tensor(out=nxt[:, k0:k1, 0:w2],
                              in0=last[:, k0:k1, 0:w2],
                              in1=last[:, k0:k1, w2:width], op=Alu.add)
        last, width = nxt, w2
    st3 = st["all"][:, :, :]
    outH = _ap(st3, 15 * GM * m, [[15, GM], [1, 9]])
    V.tensor_reduce(out=outH, in_=last[:, 0:NH, 0:width], axis=AX.X,
                    op=Alu.add)
    outC = _ap(st3, 15 * GM * m + 9, [[15, GM], [1, 6]])
    V.tensor_reduce(out=outC, in_=last[:, NH:NB, 0:width], axis=AX.X,
                    op=Alu.add)

    # sppqq per sub-tile: ACT square with accumulate over [p, 6L]
    for g in range(GM):
        p0 = off + 6 * L * g
        t = GM * m + g
        A.activation(out=ascr[:, 0:6 * L], in_=in_sb[:, p0:p0 + 6 * L],
                     func=Act.Square,
                     accum_out=st["ss"][:, t:t + 1])


class P2:
    """Emit elementwise phase-2 ops on [128, C] column tiles."""

    def __init__(self, tc, pool, c0, c1, chunk, dma_out=None):
        self.nc = tc.nc
        self.pool = pool
        self.c0, self.c1 = c0, c1
        self.C = c1 - c0
        self.chunk = chunk
        self.ctr = 0
        self.dma_out = dma_out

    def mk(self, name=None):
        self.ctr += 1
        tag = f"c{self.chunk}_" + (name or f"t{self.ctr}")
        return self.pool.tile([128, self.C], F32, tag=tag, name=tag)

    def tt(self, a, b, op, eng=None, out=None):
        dst = out if out is not None else self.mk()
        (eng or self.nc.vector).tensor_tensor(out=dst, in0=a, in1=b, op=op)
        return dst

    def mul(self, a, b, eng=None, out=None):
        return self.tt(a, b, Alu.mult, eng, out)

    def add(self, a, b, eng=None, out=None):
        return self.tt(a, b, Alu.add, eng, out)

    def sub(self, a, b, eng=None, out=None):
        return self.tt(a, b, Alu.subtract, eng, out)

    def ts(self, a, s1, op0, s2=None, op1=Alu.bypass, eng=None, out=None):
        dst = out if out is not None else self.mk()
        (eng or self.nc.vector).tensor_scalar(
            out=dst, in0=a, scalar1=s1, scalar2=s2, op0=op0, op1=op1)
        return dst

    def stt(self, a, s, b, op0, op1, eng=None, out=None):
        """(a op0 s) op1 b in one instruction."""
        dst = out if out is not None else self.mk()
        (eng or self.nc.vector).scalar_tensor_tensor(
            out=dst, in0=a, scalar=s, in1=b, op0=op0, op1=op1)
        return dst

    def recip(self, a, out=None):
        dst = out if out is not None else self.mk()
        self.nc.vector.reciprocal(out=dst, in_=a)
        return dst

    def act(self, a, func, bias=0.0, scale=1.0, out=None):
        dst = out if out is not None else self.mk()
        self.nc.scalar.activation(out=dst, in_=a, func=func, bias=bias,
                                  scale=scale)
        return dst


def _phase2(tc, p2, st, cst, loss_out):
    """Per-sample Kabsch loss from stats, columns [c0, c1) (c = sub-tile).

    t = lam + 4*wx(lam)/p'(lam); lam = s1+s2+d*s3 via Cardano on K = Hc^T Hc;
    p'(lam) = 8(s2+d*s3)(s1+d*s3)(s1+s2); wx(lam) = -|a|^2 lam^2 + Wb lam + Wc
    is the adjugate-row-0 dot product, coefficients lam-free (computed early,
    off the critical path).
    """
    nc = tc.nc
    V, G, A = nc.vector, nc.gpsimd, nc.scalar
    c0, C = p2.c0, p2.C

    St = st["all"][:, :, :]        # [p, 32, 15]
    H9 = _ap(St, 15 * c0, [[15, C], [3, 3], [1, 3]])     # [p, c, i, j]
    sp_b = _ap(St, 15 * c0 + 9, [[15, C], [1, 3], [0, 3]])
    ss = st["ss"][:, c0:c0 + C]
    invn = cst[:, c0:c0 + C]
    invn3 = cst[:, N_TILES + c0:N_TILES + c0 + C]

    def wide(name, k):
        tag = f"c{p2.chunk}_{name}"
        return p2.pool.tile([128, C * k], F32, tag=tag, name=tag)

    # spqn = spq * invn (6-wide); corr = sum(spq*spqn); ppqqc = ss - corr
    spq6 = _ap(St, 15 * c0 + 9, [[15, C], [1, 6]])
    spqn6 = wide("spqn6", 6)
    spqn6_v = spqn6[:, :].rearrange("p (c k) -> p c k", k=6)
    inb6 = invn[:, :].unsqueeze(2).broadcast_to([128, C, 6])
    V.tensor_tensor(out=spqn6_v, in0=spq6, in1=inb6, op=Alu.mult)
    corrp = wide("corrp", 6)
    corrp_v = corrp[:, :].rearrange("p (c k) -> p c k", k=6)
    G.tensor_tensor(out=corrp_v, in0=spq6, in1=spqn6_v, op=Alu.mult)
    corr = p2.mk("corr")
    V.tensor_reduce(out=corr, in_=corrp_v, axis=AX.X, op=Alu.add)
    ppqqc = p2.sub(ss, corr, G)

    # centering: Hc[c, i, j] = H - sp_i * sqn_j
    mv = wide("mv", 9)
    mv_v = mv[:, :].rearrange("p (c i j) -> p c i j", i=3, j=3)
    sqn_b = bass.AP(tensor=spqn6_v.tensor, offset=spqn6_v.offset + 3,
                    ap=[spqn6_v.ap[0], [6, C], [0, 3], [1, 3]])
    V.tensor_tensor(out=mv_v, in0=sp_b, in1=sqn_b, op=Alu.mult)
    Hc = wide("Hc", 9)
    Hc_v = Hc[:, :].rearrange("p (c k) -> p c k", k=9)
    H9f = _ap(St, 15 * c0, [[15, C], [1, 9]])
    V.tensor_tensor(out=Hc_v, in0=H9f, in1=mv[:, :].rearrange(
        "p (c k) -> p c k", k=9), op=Alu.subtract)
    hc0 = Hc[:, :]
    h = {(i, j): _ap(hc0, 3 * i + j, [[9, C]]) for i in range(3)
         for j in range(3)}

    # K = Hc^T Hc: 3 products into one (c,a,b,i) tile, single reduce
    Kt = wide("Kt", 9)
    kp = wide("kp", 27)
    for aa in range(3):
        in0 = _ap(hc0, aa, [[9, C], [0, 3], [3, 3]])
        in1 = _ap(hc0, 0, [[9, C], [1, 3], [3, 3]])
        kp_v = _ap(kp[:, :], 9 * aa, [[27, C], [3, 3], [1, 3]])
        V.tensor_tensor(out=kp_v, in0=in0, in1=in1, op=Alu.mult)
    kp_flat = _ap(kp[:, :], 0, [[3, 9 * C], [1, 3]])
    V.tensor_reduce(out=Kt[:, :], in_=kp_flat, axis=AX.X, op=Alu.add)
    trK = p2.mk("trK")
    diag_v = _ap(Kt[:, :], 0, [[9, C], [4, 3]])
    V.tensor_reduce(out=trK, in_=diag_v, axis=AX.X, op=Alu.add)
    k2 = wide("k2", 9)
    V.tensor_tensor(out=k2[:, :], in0=Kt[:, :], in1=Kt[:, :], op=Alu.mult)
    trK2 = p2.mk("trK2")
    V.tensor_reduce(out=trK2, in_=k2[:, :].rearrange("p (c k) -> p c k", k=9),
                    axis=AX.X, op=Alu.add)

    # detH (of Hc) via 2x2 minors (Pool, off-spine)
    def minor2(pq, qq, rq_, sq_, eng=G):
        t1 = p2.mul(pq, qq, eng)
        t2 = p2.mul(rq_, sq_, eng)
        return p2.sub(t1, t2, eng)

    mm1 = minor2(h[(1, 1)], h[(2, 2)], h[(1, 2)], h[(2, 1)])
    mm2 = minor2(h[(1, 0)], h[(2, 2)], h[(1, 2)], h[(2, 0)])
    mm3 = minor2(h[(1, 0)], h[(2, 1)], h[(1, 1)], h[(2, 0)])
    dd1 = p2.mul(h[(0, 0)], mm1, G)
    dd2 = p2.mul(h[(0, 1)], mm2, G)
    dd3 = p2.mul(h[(0, 2)], mm3, G)
    detH = p2.add(p2.sub(dd1, dd2, G), dd3, G)
    sgn = p2.act(detH, Act.Sign)
    detK = p2.act(detH, Act.Square)

    # --- Cardano spine starts (DVE), W-coefficient work interleaved into
    # the spine's dependency-stall windows ---
    trKsq = p2.mul(trK, trK, V)
    p6 = p2.stt(trKsq, -1.0 / 3.0, trK2, Alu.mult, Alu.add, V)
    p6c = p2.ts(p6, 1e-12, Alu.max, eng=V)
    sqp = p2.act(p6c, Act.Sqrt, scale=1.0 / 6.0)         # sqrt(p)
    mmean = p2.ts(trK, 1.0 / 3.0, Alu.mult, eng=V)

    # [fill] Horn matrix entries of M = Hc^T, packed for one-shot squares
    npk1 = p2.pool.tile([128, 3 * C], F32, tag=f"c{p2.chunk}_npk1",
                        name="npk1")
    npk2 = p2.pool.tile([128, 3 * C], F32, tag=f"c{p2.chunk}_npk2",
                        name="npk2")
    n01 = p2.sub(h[(2, 1)], h[(1, 2)], G, out=npk1[:, 0:C])
    n02 = p2.sub(h[(0, 2)], h[(2, 0)], G, out=npk1[:, C:2 * C])
    n03 = p2.sub(h[(1, 0)], h[(0, 1)], G, out=npk1[:, 2 * C:3 * C])
    n23 = p2.add(h[(2, 1)], h[(1, 2)], V, out=npk2[:, 0:C])
    n13 = p2.add(h[(0, 2)], h[(2, 0)], V, out=npk2[:, C:2 * C])
    n12 = p2.add(h[(1, 0)], h[(0, 1)], V, out=npk2[:, 2 * C:3 * C])

    msq = p2.ts(trKsq, 1.0 / 9.0, Alu.mult, eng=V)
    m3c = p2.mul(msq, mmean, G)
    u = p2.stt(detK, 0.5, m3c, Alu.mult, Alu.add, V)     # m^3 + detK/2
    tdiff = p2.sub(trKsq, trK2, V)                       # 2*M2
    tm = p2.mul(tdiff, mmean, V)
    q = p2.stt(tm, -0.25, u, Alu.mult, Alu.add, V)
    p6sq = p2.mul(p6c, p6c, V)
    p3 = p2.mul(p6sq, p6c, V)
    q2 = p2.mul(q, q, V)
    pfloor = p2.ts(p3, 9.26e-11, Alu.mult, 1e-38, Alu.max, eng=V)
    diff = p2.stt(p3, 1.0 / 216.0, q2, Alu.mult, Alu.subtract, V)
    diffc = p2.tt(diff, pfloor, Alu.max, V)
    sqd = p2.act(diffc, Act.Sqrt)

    # [fill] squares of the packed entries + first W terms
    usq = p2.pool.tile([128, 3 * C], F32, tag=f"c{p2.chunk}_usq", name="usq")
    A.activation(out=usq[:, :], in_=npk1[:, :], func=Act.Square)
    u1, u2, u3 = usq[:, 0:C], usq[:, C:2 * C], usq[:, 2 * C:3 * C]
    wsq = p2.pool.tile([128, 3 * C], F32, tag=f"c{p2.chunk}_wsq", name="wsq")
    A.activation(out=wsq[:, :], in_=npk2[:, :], func=Act.Square)
    n23s, n13s, n12s = wsq[:, 0:C], wsq[:, C:2 * C], wsq[:, 2 * C:3 * C]
    tr3 = p2.add(p2.add(h[(0, 0)], h[(1, 1)], V), h[(2, 2)], V)
    n11 = p2.stt(h[(0, 0)], 2.0, tr3, Alu.mult, Alu.subtract, V)
    n22 = p2.stt(h[(1, 1)], 2.0, tr3, Alu.mult, Alu.subtract, V)
    n33 = p2.stt(h[(2, 2)], 2.0, tr3, Alu.mult, Alu.subtract, V)
    v1 = p2.mul(n01, n02, G)
    v2 = p2.mul(n01, n03, G)
    v3 = p2.mul(n02, n03, G)

    rq = p2.recip(sqd)
    ratio = p2.mul(q, rq, V)
    ratioc = p2.ts(ratio, 100.0, Alu.min, -100.0, Alu.max, V)
    at = p2.act(ratioc, Act.Arctan)

    # [fill] Wa, Wb
    Wa_n = p2.add(p2.add(u1, u2, V), u3, V)
    s1s = p2.add(n22, n33, G)
    s2s = p2.add(n11, n33, G)
    s3s = p2.add(n11, n22, G)
    b1 = p2.mul(u1, s1s, V)
    b2 = p2.mul(u2, s2s, V)
    b3 = p2.mul(u3, s3s, V)
    b4 = p2.mul(v1, n12, G)
    b5 = p2.mul(v2, n13, G)
    b6 = p2.mul(v3, n23, G)

    # cos/sin of phi straight from `at` on ACT (func(scale*x+bias)):
    # stays on the ACT queue, no DVE round-trip
    cphi = p2.act(at, Act.Sin, bias=PI / 6.0 + PI / 2.0, scale=-1.0 / 3.0)
    sphi = p2.act(at, Act.Sin, bias=PI / 6.0, scale=-1.0 / 3.0)

    # [fill] Wb finish, Wc terms
    a123 = p2.add(p2.add(b1, b2, V), b3, V)
    c456 = p2.add(p2.add(b4, b5, G), b6, G)
    Wb = p2.stt(c456, -2.0, a123, Alu.mult, Alu.add, V)
    M1 = p2.sub(p2.mul(n22, n33, G), n23s, G)
    M2m = p2.sub(p2.mul(n11, n33, G), n13s, G)
    M3m = p2.sub(p2.mul(n11, n22, G), n12s, G)

    # eigenvalues via mp +/- sqrt(3)*ps; one packed Sqrt for all three
    pc = p2.mul(sqp, cphi, V)
    ps = p2.mul(sqp, sphi, V)
    lamp = p2.pool.tile([128, 3 * C], F32, tag=f"c{p2.chunk}_lamp",
                        name="lamp")
    p2.stt(pc, 2.0, mmean, Alu.mult, Alu.add, V, out=lamp[:, 0:C])
    mp = p2.sub(mmean, pc, V)
    s3p = p2.ts(ps, SQ3, Alu.mult, eng=V)
    lam2 = p2.add(mp, s3p, V)
    p2.ts(lam2, 0.0, Alu.max, eng=V, out=lamp[:, C:2 * C])
    lam3 = p2.sub(mp, s3p, V)
    p2.ts(lam3, 0.0, Alu.max, eng=V, out=lamp[:, 2 * C:3 * C])
    sgt = p2.pool.tile([128, 3 * C], F32, tag=f"c{p2.chunk}_sgt", name="sgt")
    A.activation(out=sgt[:, :], in_=lamp[:, :], func=Act.Sqrt)
    sg1 = sgt[:, 0:C]
    sg2 = sgt[:, C:2 * C]
    sg3 = sgt[:, 2 * C:3 * C]

    # [fill] Wc finish
    dd_ = p2.add(p2.add(p2.mul(u1, M1, G), p2.mul(u2, M2m, G), G),
                 p2.mul(u3, M3m, G), G)
    cc1 = p2.sub(p2.mul(n12, n33, V), p2.mul(n13, n23, V), V)
    cc2 = p2.sub(p2.mul(n12, n23, V), p2.mul(n13, n22, V), V)
    cc3 = p2.sub(p2.mul(n11, n23, V), p2.mul(n12, n13, V), V)
    ee = p2.add(p2.sub(p2.mul(v1, cc1, V), p2.mul(v2, cc2, V), V),
                p2.mul(v3, cc3, V), V)
    Wc = p2.stt(ee, 2.0, dd_, Alu.mult, Alu.subtract, V)   # 2*ee - dd

    s3d = p2.mul(sgn, sg3, V)
    t12 = p2.add(sg1, sg2, V)
    lam = p2.add(t12, s3d, V)                            # lambda_max of Horn
    pp1 = p2.add(sg2, s3d, V)
    pp2 = p2.add(sg1, s3d, V)
    ppr = p2.mul(pp1, pp2, V)
    ppr2 = p2.mul(ppr, t12, V)                           # p'(lam)/8
    pprc = p2.ts(ppr2, 1e-13, Alu.max, eng=V)
    rp = p2.recip(pprc)

    # wx = (-Wa_n*lam + Wb)*lam + Wc, then t and the loss
    wt1 = p2.mul(Wa_n, lam, V)
    wt2 = p2.sub(Wb, wt1, V)
    wt3 = p2.mul(wt2, lam, V)
    wx_v = p2.add(wt3, Wc, V)
    corr4 = p2.mul(wx_v, rp, V)
    t_unc = p2.stt(corr4, 0.5, lam, Alu.mult, Alu.add, V)  # lam + 4wx/p'
    ssum = p2.add(t12, sg3, V)
    tb = p2.tt(t_unc, ssum, Alu.min, V)
    ssn = p2.ts(ssum, -1.0, Alu.mult, eng=V)
    tcl = p2.tt(tb, ssn, Alu.max, V)
    li = p2.stt(tcl, -2.0, ppqqc, Alu.mult, Alu.add, V)
    p2.mul(li, invn3, V, out=loss_out)
    if p2.dma_out is not None:
        p2.dma_out(0, C)


def build_program(lmaxes, chunks=((0, 32),), n_dma=5, bH=28, bC=18,
                  wbufs=3, order="desc", bF2=0):
    """lmaxes: per-merged-tile crop lengths (16 ints, multiples of 4)."""
    assert len(lmaxes) == N_MERGED
    tot = sum(6 * GM * L for L in lmaxes)
    offs = []
    o = 0
    for L in lmaxes:
        offs.append(o)
        o += 6 * GM * L

    nc = bass.Bass("TRN2", debug=False, enable_asserts=False,
                   target_bir_lowering=False)
    # extra activation-bias constants (only 0.0/1.0 pre-registered)
    for cval in (PI / 2.0, PI / 6.0 + PI / 2.0, PI / 6.0):
        cten = nc.alloc_sbuf_tensor(f"const-f32-{cval}", [128, 1], F32)
        nc.gpsimd.memset(cten.ap(), cval)
        nc.const_aps.aps[(F32, cval)] = cten.ap()
    nc.all_engine_barrier()
    pq = nc.dram_tensor("pq", [128, tot], BF16, kind="ExternalInput").ap()
    cstd = nc.dram_tensor("cst", [128, 2 * N_TILES], F32,
                          kind="ExternalInput").ap()
    loss = nc.dram_tensor("loss", [128, N_TILES], F32,
                          kind="ExternalOutput").ap()

    with tile.TileContext(nc) as tc:
        from contextlib import ExitStack
        with ExitStack() as ctx:
            pools = {
                "in": ctx.enter_context(tc.tile_pool(name="inp", bufs=1)),
                "work": ctx.enter_context(tc.tile_pool(name="work", bufs=wbufs)),
                "scr": ctx.enter_context(tc.tile_pool(name="scr", bufs=3)),
                "stats": ctx.enter_context(tc.tile_pool(name="stats", bufs=1)),
                "ph2": ctx.enter_context(tc.tile_pool(name="ph2", bufs=1)),
            }
            pools["bH"] = bH
            pools["bC"] = bC
            pools["bF2"] = bF2
            in_sb = pools["in"].tile([128, tot], BF16, tag="in", name="in")
            cst = pools["stats"].tile([128, 2 * N_TILES], F32, tag="cst",
                                      name="cst")
            st = {
                "all": pools["stats"].tile([128, N_TILES, 15], F32,
                                           tag="st_all", name="st_all"),
                "ss": pools["stats"].tile([128, N_TILES], F32,
                                          tag="st_ss", name="st_ss"),
            }
            loss_tile = pools["ph2"].tile([128, N_TILES], F32, tag="loss",
                                          name="loss")

            # input DMAs: small first chunk so compute starts early, then
            # n_dma-1 even chunks over the rest; cst after the first chunk
            bounds = [0, 1]
            rem = N_MERGED - 1
            for d in range(n_dma - 1):
                bounds.append(1 + ((d + 1) * rem) // (n_dma - 1))
            first = True
            for ma, mb in zip(bounds[:-1], bounds[1:]):
                if ma >= mb:
                    continue
                e0 = offs[ma]
                e1 = offs[mb - 1] + 6 * GM * lmaxes[mb - 1]
                if first:
                    # stage the first chunk: one sub-tile, rest of tile, rest
                    eq = e0 + 6 * lmaxes[ma]
                    eh = e0 + 6 * GM * lmaxes[ma]
                    nc.sync.dma_start(out=in_sb[:, e0:eq], in_=pq[:, e0:eq])
                    nc.sync.dma_start(out=in_sb[:, eq:eh], in_=pq[:, eq:eh])
                    if eh < e1:
                        nc.sync.dma_start(out=in_sb[:, eh:e1],
                                          in_=pq[:, eh:e1])
                    first = False
                else:
                    nc.sync.dma_start(out=in_sb[:, e0:e1], in_=pq[:, e0:e1])
            # constants are first needed by phase 2 -> keep them out of the
            # ramp-critical part of the input stream
            nc.sync.dma_start(out=cst[:, :], in_=cstd)

            ci = 0
            prods = {}
            prods[0] = _emit_products(tc, pools, in_sb, 0, lmaxes[0], offs[0])
            for m in range(N_MERGED):
                if m + 1 < N_MERGED:
                    prods[m + 1] = _emit_products(
                        tc, pools, in_sb, m + 1, lmaxes[m + 1], offs[m + 1])
                _phase1_rest(tc, pools, in_sb, st, m, lmaxes[m], offs[m],
                             prods.pop(m))
                while ci < len(chunks) and GM * (m + 1) >= chunks[ci][1]:
                    a, b = chunks[ci]

                    def _dma_out(x0, x1, a=a):
                        nc.sync.dma_start(out=loss[:, a + x0:a + x1],
                                          in_=loss_tile[:, a + x0:a + x1])
                    p2 = P2(tc, pools["ph2"], a, b, ci, dma_out=_dma_out)
                    _phase2(tc, p2, st, cst, loss_tile[:, a:b])
                    ci += 1
    _legalize_single_wait(nc)
    return nc


_nc_cache = {}


def _get_program(lmaxes, chunks=((0, 32),), n_dma=5, bH=28, bC=18, wbufs=3,
                 order="desc"):
    key = (lmaxes, chunks, n_dma, bH, bC, wbufs, order)
    if key not in _nc_cache:
        _nc_cache[key] = build_program(lmaxes, chunks, n_dma, bH, bC, wbufs,
                                       order)
    return _nc_cache[key]


def _prep(pred_coord, true_coord, pad_mask, torder="desc"):
    """Host-side packing. Returns (lmaxes, in_maps)."""
    P = np.asarray(pred_coord, dtype=np.float32)
    Q = np.asarray(true_coord, dtype=np.float32)
    M = np.asarray(pad_mask)
    B = P.shape[0]
    assert B == B_FULL and P.shape[1] == N_SEQ

    lengths = (N_SEQ - M.sum(axis=1)).astype(np.int64)
    order = np.argsort(lengths, kind="stable")
    lsort = lengths[order]
    # merged tile m takes sorted block blk[m]; longest first so the early
    # DMA chunks carry the most compute
    if torder == "ilv":
        blk = []
        hi, lo = N_MERGED - 1, N_MERGED // 2 - 1
        for i in range(N_MERGED // 2):
            blk.append(hi - i)
            blk.append(lo - i)
        blk = tuple(blk)
    else:
        blk = tuple(range(N_MERGED - 1, -1, -1))
    bsz = 1024 * GM
    lmaxes = []
    for m in range(N_MERGED):
        L = int(lsort[bsz * (blk[m] + 1) - 1])
        L = max(8, (L + 7) & ~7)
        lmaxes.append(L)
    lmaxes = tuple(lmaxes)

    # zero padding, transpose to [B, 3, N], sort
    w = (np.arange(N_SEQ)[None, :] < lengths[:, None]).astype(np.float32)
    Pz = (P * w[:, :, None]).transpose(0, 2, 1)[order]   # [B, 3, N]
    Qz = (Q * w[:, :, None]).transpose(0, 2, 1)[order]
    Pb = Pz.astype(ml_dtypes.bfloat16)
    Qb = Qz.astype(ml_dtypes.bfloat16)

    tot = sum(6 * GM * L for L in lmaxes)
    in_maps = []
    linv = (1.0 / lsort.astype(np.float64)).astype(np.float32)
    for c in range(N_CORES):
        buf = np.zeros((128, tot), dtype=ml_dtypes.bfloat16)
        o = 0
        for m in range(N_MERGED):
            L = lmaxes[m]
            bm = blk[m]
            gsel = np.arange(bsz * bm + c, bsz * (bm + 1), 8)  # 128*GM sorted
            Pm = Pb[gsel][:, :, :L]      # [128*GM, 3, L]
            Qm = Qb[gsel][:, :, :L]
            for g in range(GM):
                sl = slice(128 * g, 128 * (g + 1))
                buf[:, o:o + 3 * L] = Pm[sl].reshape(128, 3 * L)
                buf[:, o + 3 * L:o + 6 * L] = Qm[sl].reshape(128, 3 * L)
                o += 6 * L
        # constants: invn (32 cols), invn/3 (32 cols); col t, partition p
        # -> sorted index (t*128+p)*8 + c
        idx = (np.arange(B_CORE) * 8 + c)
        nin = linv[idx].reshape(N_TILES, 128).T          # [128, 32]
        # column t = sub-tile GM*m+g holds sorted sub-block GM*blk[m]+g
        perm = [GM * blk[t // GM] + (t % GM) for t in range(N_TILES)]
        nin = nin[:, perm]
        cstv = np.concatenate([nin, nin / 3.0], axis=1).astype(np.float32)
        in_maps.append({"pq": buf, "cst": np.ascontiguousarray(cstv)})
    return lmaxes, in_maps


def kernel(pred_coord, true_coord, pad_mask):
    lmaxes, in_maps = _prep(pred_coord, true_coord, pad_mask)
    nc = _get_program(lmaxes)
    trace = bool(int(os.environ.get("KERNEL_TRACE", "0")))
    res = run_bass_kernel_spmd(nc, in_maps, core_ids=list(range(N_CORES)),
                               trace=trace)
    if trace and res.exec_time_ns is not None:
        print(f"HW exec time: {res.exec_time_ns} ns")
        kernel.last_exec_time_ns = res.exec_time_ns
    total = 0.0
    for r in res.results:
        total += r["loss"].astype(np.float64).sum()
    return np.float32(total / B_FULL)


kernel.last_exec_time_ns = None
